# revision 1
# baseline (speedup 1.0000x reference)
"""SlimMambaBlock Trainium2 kernel.

Full-input contract: kernel(**inputs) takes the complete tensors
(x [8, 4096, 256], norm_w [256], W_in [1024, 256], W_dt [512, 512],
b_dt [512], W_out [256, 512]) and returns the full output [8, 4096, 256].

Sharding: data-parallel over batch — core b processes batch b (8 cores).

Per-core program (Tile framework), feature-major activation layout:
  1. RMSNorm x -> h. inv_rms via DVE Newton-rsqrt (bit-trick seed +
     3 iterations) so the ScalarE only ever needs ONE activation table
     set (silu_and_others = {Silu, Tanh, Square, Copy, ...}) — table
     reloads are 1.3us each and would otherwise dominate.
  2. PE-transpose h tiles -> hT [d, tok]
  3. in_proj: uvT[feat, tok] = W_inT.T @ hT ; u = silu, g = silu (ACT)
  4. dt_proj: preT = W_dtT.T @ uT ; th = tanh(pre/2 + b_dt/2) (ACT)
     lam = sigmoid(pre + b_dt) = 0.5*th + 0.5 (DVE tensor_scalar)
  5. recurrence via DVE tensor_tensor_scan along the time (free) axis,
     with S = -2*s (change of variable so data1 = (th-1)*u needs no
     extra scaling):  S_t = lam_t * S_{t-1} + (th_t - 1) * u_t
  6. sg = s*g = (S * -0.5) * g   (gpsimd)
  7. out_proj with sgT as the stationary operand: y[tok, d] = sgT.T @ W_outT
  8. out = x + y (DVE, PSUM+SBUF), DMA out.

Matmul operands are bf16 (PE 1 cycle/row vs 4 for fp32; float32r is
rejected by walrus on non-PE engines). The recurrence runs in bf16
tensors with the scan's fp32 internal state; RMS stats, Newton-rsqrt,
and the final residual add stay fp32. Measured rel err ~8e-4.
"""

import numpy as np

B, K, D = 8, 4096, 256
INNER = 512
EPS = 1e-5
TC = 512                 # tokens per chunk
NCHUNK = K // TC         # 8
NTT = TC // 128          # token-tiles per chunk

N_CORES = 8
MAGIC = 0x5F3759DF       # fast inverse sqrt seed

_CACHE: dict = {}


def _emit(tc, aps, mm_f32r=True, silu_native=True):
    """Emit the per-core program. aps: dict of DRAM APs."""
    import concourse.bass as bass
    import concourse.mybir as mybir
    from concourse import masks

    nc = tc.nc
    f32 = mybir.dt.float32
    f32r = mybir.dt.float32r
    i32 = mybir.dt.int32
    AF = mybir.ActivationFunctionType
    ALU = mybir.AluOpType
    ts = bass.ts

    # Fast mode: all matmul operands in bf16 (PE runs 1 cycle/row vs 4 for
    # fp32; bf16 is encodable on every engine, unlike float32r which walrus
    # rejects outside the PE). The recurrence (tanh/lam/b/scan state) stays
    # fp32 for precision; only matmul inputs are rounded.
    fr = mybir.dt.bfloat16 if mm_f32r else f32

    def mm(ap):
        return ap

    def pe_transpose(out_ap, in_ap, ident_ap):
        nc.tensor.matmul(out_ap, in_ap, ident_ap, is_transpose=True)

    x_d = aps["x"]
    nw_d = aps["norm_w"]
    win_d = aps["W_in"]
    wdt_d = aps["W_dt"]
    bdt_d = aps["b_dt"]
    wout_d = aps["W_out"]
    out_d = aps["out"]

    import contextlib
    ctx = contextlib.ExitStack()
    with ctx:
        const = ctx.enter_context(tc.tile_pool(name="const", bufs=1))
        wraw = ctx.enter_context(tc.tile_pool(name="wraw", bufs=1))
        wT = ctx.enter_context(tc.tile_pool(name="wT", bufs=1))
        xp = ctx.enter_context(tc.tile_pool(name="xp", bufs=3 * NTT))
        sqp = ctx.enter_context(tc.tile_pool(name="sqp", bufs=2))
        statp = ctx.enter_context(tc.tile_pool(name="statp", bufs=6))
        hp = ctx.enter_context(tc.tile_pool(name="hp", bufs=2 * NTT))
        hTp = ctx.enter_context(tc.tile_pool(name="hTp", bufs=2))
        uTp = ctx.enter_context(tc.tile_pool(name="uTp", bufs=2))
        gTp = ctx.enter_context(tc.tile_pool(name="gTp", bufs=2))
        thTp = ctx.enter_context(tc.tile_pool(name="thTp", bufs=2))
        lamTp = ctx.enter_context(tc.tile_pool(name="lamTp", bufs=2))
        bTp = ctx.enter_context(tc.tile_pool(name="bTp", bufs=2))
        sTp = ctx.enter_context(tc.tile_pool(name="sTp", bufs=2))
        sgTp = ctx.enter_context(tc.tile_pool(name="sgTp", bufs=2))
        outp = ctx.enter_context(tc.tile_pool(name="outp", bufs=2 * NTT))

        tps = ctx.enter_context(tc.tile_pool(name="tps", bufs=2, space="PSUM"))
        uvps = ctx.enter_context(tc.tile_pool(name="uvps", bufs=2, space="PSUM"))
        preps = ctx.enter_context(tc.tile_pool(name="preps", bufs=2, space="PSUM"))
        yps = ctx.enter_context(tc.tile_pool(name="yps", bufs=2, space="PSUM"))

        # ---- constants ----
        identf = const.tile([128, 128], f32, tag="identf", name="identf")
        masks.make_identity(nc, identf[:])
        ident = identf
        magic = const.tile([128, NTT], i32, tag="magic", name="magic")
        nc.gpsimd.memset(magic[:], MAGIC)

        nw = []
        for k in range(2):
            t = const.tile([128, 1], f32, tag=f"nw{k}", name=f"nw{k}")
            nc.sync.dma_start(t[:], nw_d[ts(k, 128)].rearrange("(a b) -> a b", b=1))
            nw.append(t)
        bdt2 = []
        for m in range(4):
            t = const.tile([128, 1], f32, tag=f"bdt{m}", name=f"bdt{m}")
            nc.sync.dma_start(t[:], bdt_d[ts(m, 128)].rearrange("(a b) -> a b", b=1))
            # scale in place: tanh(pre*0.5 + b_dt*0.5)
            nc.vector.tensor_scalar_mul(t[:], t[:], 0.5)
            bdt2.append(t)

        # ---- load + transpose weights ----
        # W_in [1024(feat), 256(d)] -> W_inT [2][128(d), 1024(feat)] * norm_w
        win_raw = []
        for f in range(8):
            t = wraw.tile([128, 256], f32, tag=f"winr{f}", name=f"winr{f}")
            nc.sync.dma_start(t[:], win_d[ts(f, 128), :])
            win_raw.append(t)
        winT = []
        for k in range(2):
            t = wT.tile([128, 1024], fr, tag=f"winT{k}", name=f"winT{k}")
            winT.append(t)
        for k in range(2):
            for half in range(2):
                p = tps.tile([128, 512], f32, tag="tp", name="tpw")
                for j in range(4):
                    f = half * 4 + j
                    pe_transpose(p[:, ts(j, 128)], win_raw[f][:, ts(k, 128)],
                                 identf[:])
                if half % 2 == 0:
                    nc.vector.tensor_copy(winT[k][:, ts(half, 512)], p[:])
                else:
                    nc.scalar.copy(winT[k][:, ts(half, 512)], p[:])
        for k in range(2):
            # fold norm_w (per-d row scale) into W_inT
            nc.vector.tensor_scalar_mul(winT[k][:], winT[k][:], nw[k][:])

        # W_dt [512(e_out), 512(e_in)] -> W_dtT [4][128(e_in), 512(e_out)]
        wdt_raw = []
        for m in range(4):
            t = wraw.tile([128, 512], f32, tag=f"wdtr{m}", name=f"wdtr{m}")
            nc.sync.dma_start(t[:], wdt_d[ts(m, 128), :])
            wdt_raw.append(t)
        wdtT = []
        for k in range(4):
            t = wT.tile([128, 512], fr, tag=f"wdtT{k}", name=f"wdtT{k}")
            wdtT.append(t)
        for k in range(4):
            p = tps.tile([128, 512], f32, tag="tp", name="tpw")
            for m in range(4):
                pe_transpose(p[:, ts(m, 128)], wdt_raw[m][:, ts(k, 128)],
                             identf[:])
            if k % 2 == 0:
                nc.vector.tensor_copy(wdtT[k][:], p[:])
            else:
                nc.scalar.copy(wdtT[k][:], p[:])

        # W_out [256(d), 512(e)] -> W_outT [4][128(e), 256(d)]
        wout_raw = []
        for dd in range(2):
            t = wraw.tile([128, 512], f32, tag=f"woutr{dd}", name=f"woutr{dd}")
            nc.sync.dma_start(t[:], wout_d[ts(dd, 128), :])
            wout_raw.append(t)
        woutT = []
        for e in range(4):
            t = wT.tile([128, 256], fr, tag=f"woutT{e}", name=f"woutT{e}")
            woutT.append(t)
        for e in range(4):
            p = tps.tile([128, 512], f32, tag="tp", name="tpw")
            for dd in range(2):
                pe_transpose(p[:, ts(dd, 128)], wout_raw[dd][:, ts(e, 128)],
                             identf[:])
            if e % 2 == 0:
                nc.vector.tensor_copy(woutT[e][:], p[:, :256])
            else:
                nc.scalar.copy(woutT[e][:], p[:, :256])

        # ---- main chunk loop ----
        def norm_stage(c):
            """Load x chunk, RMS stats, h = x*inv_rms, PE-transpose -> hT.
            Returns (xts, hT)."""
            xts = []
            vpk = statp.tile([128, NTT], f32, tag="vpk", name="vpk")
            for t in range(NTT):
                xt = xp.tile([128, D], f32, tag="xt", name="xt")
                nc.sync.dma_start(xt[:], x_d[ts(c * NTT + t, 128), :])
                xts.append(xt)
                sq = sqp.tile([128, D], f32, tag="sq", name="sq")
                nc.scalar.activation(sq[:], xt[:], AF.Square,
                                     accum_out=vpk[:, t:t + 1])

            # inv_rms = rsqrt(vpk/D + eps) via Newton on DVE (packed [128,4])
            nv = statp.tile([128, NTT], f32, tag="nv", name="nv")
            nc.vector.tensor_scalar(nv[:], vpk[:], 1.0 / D, EPS,
                                    op0=ALU.mult, op1=ALU.add)
            ny = statp.tile([128, NTT], f32, tag="ny", name="ny")
            # seed: y0 = bits(magic - (bits(v) >> 1))
            nyi = ny[:].bitcast(i32)
            nc.vector.tensor_scalar(nyi, nv[:].bitcast(i32), 1, None,
                                    op0=ALU.arith_shift_right)
            nc.vector.scalar_tensor_tensor(nyi, magic[:], 1, nyi,
                                           op0=ALU.bypass, op1=ALU.subtract)
            nt = statp.tile([128, NTT], f32, tag="nt", name="nt")
            for _ in range(3):
                # t = v*y*y ; y = y * (1.5 - 0.5*t)
                nc.gpsimd.tensor_mul(nt[:], ny[:], ny[:])
                nc.gpsimd.tensor_mul(nt[:], nt[:], nv[:])
                nc.vector.tensor_scalar(nt[:], nt[:], -0.5, 1.5,
                                        op0=ALU.mult, op1=ALU.add)
                nc.gpsimd.tensor_mul(ny[:], ny[:], nt[:])

            # h = x * inv_rms ; PE-transpose into hT (batched per d-half:
            # 4 transposes into one PSUM bank, then one wide copy)
            hT = [hTp.tile([128, TC], fr, tag=f"hT{k}", name=f"hT{k}")
                  for k in range(2)]
            hs = []
            for t in range(NTT):
                h = hp.tile([128, D], f32, tag="h", name="h")
                nc.gpsimd.tensor_scalar_mul(h[:], xts[t][:], ny[:, t:t + 1])
                hs.append(h)
            for k in range(2):
                p = tps.tile([128, TC], f32, tag="tp", name="tp")
                for t in range(NTT):
                    pe_transpose(p[:, ts(t, 128)], hs[t][:, ts(k, 128)],
                                 identf[:])
                nc.vector.tensor_copy(hT[k][:], p[:])
            return xts, hT

        sT_prev = None
        nxt = norm_stage(0)
        def front_stage(st):
            """in_proj+silu, dt_proj+tanh+lam, bT — the pre-scan work."""
            hT = st["hT"]
            uT = [uTp.tile([128, TC], fr, tag=f"uT{m}", name=f"uT{m}")
                  for m in range(4)]
            gT = [gTp.tile([128, TC], f32, tag=f"gT{m}", name=f"gT{m}")
                  for m in range(4)]
            for m in range(8):
                ps = uvps.tile([128, TC], f32, tag="uv", name="uv")
                for k in range(2):
                    nc.tensor.matmul(
                        ps[:], mm(winT[k][:, ts(m, 128)]), mm(hT[k][:]),
                        start=(k == 0), stop=(k == 1),
                    )
                dst = uT[m] if m < 4 else gT[m - 4]
                if silu_native:
                    nc.scalar.activation(dst[:], ps[:], AF.Silu)
                else:
                    # CoreSim has no Silu: decompose as x * sigmoid(x)
                    sig = sqp.tile([128, TC], f32, tag="sig", name="sig")
                    nc.scalar.activation(sig[:], ps[:], AF.Sigmoid)
                    nc.vector.tensor_mul(dst[:], ps[:], sig[:])

            thT = [thTp.tile([128, TC], fr, tag=f"thT{m}", name=f"thT{m}")
                   for m in range(4)]
            lamT = [lamTp.tile([128, TC], fr, tag=f"lamT{m}", name=f"lamT{m}")
                    for m in range(4)]
            for m in range(4):
                ps = preps.tile([128, TC], f32, tag="pre", name="pre")
                for k in range(4):
                    nc.tensor.matmul(
                        ps[:], mm(wdtT[k][:, ts(m, 128)]), mm(uT[k][:]),
                        start=(k == 0), stop=(k == 3),
                    )
                nc.scalar.activation(thT[m][:], ps[:], AF.Tanh,
                                     bias=bdt2[m][:], scale=0.5)
                nc.vector.tensor_scalar(lamT[m][:], thT[m][:], 0.5, 0.5,
                                        op0=ALU.mult, op1=ALU.add)

            bT = [bTp.tile([128, TC], fr, tag=f"bT{m}", name=f"bT{m}")
                  for m in range(4)]
            for m in range(4):
                tm = bTp.tile([128, TC], fr, tag="tm", name="tm")
                nc.gpsimd.tensor_scalar(tm[:], thT[m][:], 1.0, None,
                                        op0=ALU.subtract)
                nc.gpsimd.tensor_mul(bT[m][:], tm[:], uT[m][:])
            st.update(uT=uT, gT=gT, lamT=lamT, bT=bT)

        def scan_stage(st, sT_prev):
            # scan (DVE): S_t = lam_t*S_{t-1} + bT_t  => S = -2*s
            sT = [sTp.tile([128, TC], fr, tag=f"sT{m}", name=f"sT{m}")
                  for m in range(4)]
            sgT = [sgTp.tile([128, TC], fr, tag=f"sgT{m}", name=f"sgT{m}")
                   for m in range(4)]
            for m in range(4):
                init = 0.0 if sT_prev is None else sT_prev[m][:, TC - 1:TC]
                nc.vector.tensor_tensor_scan(
                    sT[m][:], st["lamT"][m][:], st["bT"][m][:], init,
                    op0=ALU.mult, op1=ALU.add,
                )
                # sg = s*g = (S * -0.5) * g, right after its scan so
                # out_proj's e-th accumulation can start immediately
                nc.vector.scalar_tensor_tensor(
                    sgT[m][:], sT[m][:], -0.5, st["gT"][m][:],
                    op0=ALU.mult, op1=ALU.mult,
                )
            st.update(sT=sT, sgT=sgT)
            return sT

        def out_stage(st, c):
            # out_proj (sgT stationary -> y in [tok, d]) + residual + store
            sgT, xts = st["sgT"], st["xts"]
            for pair in range(NTT // 2):
                tt = (2 * pair, 2 * pair + 1)
                yp2 = [yps.tile([128, D], f32, tag="y", name="y") for _ in tt]
                for e in range(4):
                    for i, t in enumerate(tt):
                        nc.tensor.matmul(
                            yp2[i][:], mm(sgT[e][:, ts(t, 128)]),
                            mm(woutT[e][:]),
                            start=(e == 0), stop=(e == 3),
                        )
                for i, t in enumerate(tt):
                    ot = outp.tile([128, D], f32, tag="ot", name="ot")
                    nc.vector.tensor_add(ot[:], yp2[i][:], xts[t][:])
                    nc.sync.dma_start(out_d[ts(c * NTT + t, 128), :], ot[:])

        # Software-pipelined emission. Engines run their streams in order,
        # so next chunk's PE-heavy front must be emitted BEFORE this
        # chunk's out_proj for PE to stay busy during the scan tail.
        sts = [dict() for _ in range(NCHUNK)]
        sts[0]["xts"], sts[0]["hT"] = norm_stage(0)
        front_stage(sts[0])
        if NCHUNK > 1:
            sts[1]["xts"], sts[1]["hT"] = norm_stage(1)
        sT_prev = None
        for c in range(NCHUNK):
            sT_prev = scan_stage(sts[c], sT_prev)
            if c + 1 < NCHUNK:
                front_stage(sts[c + 1])
            if c + 2 < NCHUNK:
                sts[c + 2]["xts"], sts[c + 2]["hT"] = norm_stage(c + 2)
            out_stage(sts[c], c)
            sts[c].clear()


def build(mm_f32r=True, silu_native=True):
    """Build and compile the Bass module (cached)."""
    key = ("nc", mm_f32r, silu_native)
    if key in _CACHE:
        return _CACHE[key]

    from concourse import bacc, mybir, tile

    f32 = mybir.dt.float32
    nc = bacc.Bacc(
        "TRN2",
        target_bir_lowering=False,
        debug=False,
        num_devices=N_CORES,
    )
    aps = {
        "x": nc.dram_tensor("x", [K, D], f32, kind="ExternalInput").ap(),
        "norm_w": nc.dram_tensor("norm_w", [D], f32, kind="ExternalInput").ap(),
        "W_in": nc.dram_tensor("W_in", [2 * INNER, D], f32, kind="ExternalInput").ap(),
        "W_dt": nc.dram_tensor("W_dt", [INNER, INNER], f32, kind="ExternalInput").ap(),
        "b_dt": nc.dram_tensor("b_dt", [INNER], f32, kind="ExternalInput").ap(),
        "W_out": nc.dram_tensor("W_out", [D, INNER], f32, kind="ExternalInput").ap(),
        "out": nc.dram_tensor("out", [K, D], f32, kind="ExternalOutput").ap(),
    }
    with tile.TileContext(nc) as tc:
        _emit(tc, aps, mm_f32r=mm_f32r, silu_native=silu_native)
    nc.compile()
    _CACHE[key] = nc
    return nc


def make_in_maps(inputs):
    x = np.asarray(inputs["x"], dtype=np.float32)
    shared = {
        "norm_w": np.asarray(inputs["norm_w"], dtype=np.float32),
        "W_in": np.asarray(inputs["W_in"], dtype=np.float32),
        "W_dt": np.asarray(inputs["W_dt"], dtype=np.float32),
        "b_dt": np.asarray(inputs["b_dt"], dtype=np.float32),
        "W_out": np.asarray(inputs["W_out"], dtype=np.float32),
    }
    return [
        {"x": np.ascontiguousarray(x[b]), **shared} for b in range(N_CORES)
    ]


def run(inputs, trace=False, mm_f32r=True, silu_native=True, **kw):
    from concourse.bass_utils import run_bass_kernel_spmd

    nc = build(mm_f32r=mm_f32r, silu_native=silu_native)
    in_maps = make_in_maps(inputs)
    res = run_bass_kernel_spmd(
        nc, in_maps, core_ids=list(range(N_CORES)), trace=trace, **kw
    )
    out = np.stack([res.results[b]["out"] for b in range(N_CORES)], axis=0)
    return out, res


def kernel(**inputs) -> np.ndarray:
    out, _ = run(inputs, trace=False)
    return out



# revision 19
# speedup vs baseline: 1.4296x; 1.4296x over previous
"""SlimMambaBlock Trainium2 kernel.

Full-input contract: kernel(**inputs) takes the complete tensors
(x [8, 4096, 256], norm_w [256], W_in [1024, 256], W_dt [512, 512],
b_dt [512], W_out [256, 512]) and returns the full output [8, 4096, 256].

Sharding: data-parallel over batch — core b processes batch b (8 cores).

Per-core program (Tile framework), feature-major activation layout.
Engine budget per 512-token chunk (cost model): PE ~9.0us (roofline);
ACT ~8.4us, DVE ~7.5us, Pool ~7.3us, SP ~3us — everything squeezed
under the matmul shadow. Engine placement is ISA-constrained: gpsimd
(Pool) has no scan / no 3-operand stt / no PSUM access on TRN2, so all
scans + PSUM drains live on DVE and Pool gets SBUF-only tensor work.

  1. RMSNorm stats via ACT Square+accum (one table set:
     silu_and_others = {Silu, Tanh, Square, Copy}; a table reload is
     1.3us). inv_rms via Newton-rsqrt entirely on Pool (bit-trick seed
     + 2 iterations, packed [128,4]).
  2. h = x*inv_rms in bf16 (Pool ts), PE-transpose (1 cyc/row at bf16)
     into ONE [128,1024] bf16 PSUM bank; one wide DVE copy (2x mode)
     -> hT [d, k-half x tok].
  3. in_proj: uvT[feat, tok] = W_inT.T @ hT ; u = silu, g = silu (ACT)
     writing m-slices of wide [128,2048] bf16 tensors.
  4. dt_proj: preT = W_dtT.T @ uT ; th = tanh(pre/2 + b_dt/2) (ACT).
     lam = 0.5*th + 0.5 and tm = th - 1: one wide 4x-mode DVE ts each.
     b = tm*u: two [128,1024] Pool tensor_muls.
  5. recurrence S_t = lam_t*S_{t-1} + b_t (S = -2*s) — 4 DVE scans
     into slices of a wide sT.
  6. sg = S*g: ONE wide [128,2048] 2x-mode DVE tensor_mul (the -0.5
     that undoes the change of variable is pre-folded into W_outT).
  7. out_proj -> PSUM [128,512] banks; residual add fused with the
     PSUM->SBUF drain as one wide scalar_tensor_tensor per half-chunk
     (DVE), then one wide DMA out per half-chunk.

Matmul operands are bf16 (PE 1 cycle/row vs 4 for fp32). RMS stats and
Newton-rsqrt stay fp32. Measured rel err ~8e-4.

build(repeat=R) emits the main loop R times inside one NEFF; test.py
uses (time(R) - time(1))/(R-1) to cancel the large per-call dispatch
overhead of the axon tunnel when timing.
"""

import numpy as np

B, K, D = 8, 4096, 256
INNER = 512
EPS = 1e-5
TC = 512                 # tokens per chunk
NCHUNK = K // TC         # 8
NTT = TC // 128          # token-tiles per chunk
WID = 4 * TC             # wide feature-major tensors: [128, 4*512]

N_CORES = 8
MAGIC = 0x5F3759DF       # fast inverse sqrt seed

_CACHE: dict = {}


def _emit(tc, aps, mm_f32r=True, silu_native=True, repeat=1):
    """Emit the per-core program. aps: dict of DRAM APs."""
    import concourse.bass as bass
    import concourse.mybir as mybir
    from concourse import masks

    nc = tc.nc
    f32 = mybir.dt.float32
    i32 = mybir.dt.int32
    AF = mybir.ActivationFunctionType
    ALU = mybir.AluOpType
    ts = bass.ts

    fr = mybir.dt.bfloat16 if mm_f32r else f32

    x_d = aps["x"]
    nw_d = aps["norm_w"]
    win_d = aps["W_in"]
    wdt_d = aps["W_dt"]
    bdt_d = aps["b_dt"]
    wout_d = aps["W_out"]
    out_d = aps["out"]

    import contextlib
    ctx = contextlib.ExitStack()
    with ctx:
        const = ctx.enter_context(tc.tile_pool(name="const", bufs=1))
        wT = ctx.enter_context(tc.tile_pool(name="wT", bufs=1))
        xp = ctx.enter_context(tc.tile_pool(name="xp", bufs=4))
        outp = ctx.enter_context(tc.tile_pool(name="outp", bufs=4))
        sqp = ctx.enter_context(tc.tile_pool(name="sqp", bufs=2))
        statp = ctx.enter_context(tc.tile_pool(name="statp", bufs=3))
        hp = ctx.enter_context(tc.tile_pool(name="hp", bufs=2 * NTT))
        hTp = ctx.enter_context(tc.tile_pool(name="hTp", bufs=2))
        uTp = ctx.enter_context(tc.tile_pool(name="uTp", bufs=2))
        gTp = ctx.enter_context(tc.tile_pool(name="gTp", bufs=2))
        thTp = ctx.enter_context(tc.tile_pool(name="thTp", bufs=2))
        lamTp = ctx.enter_context(tc.tile_pool(name="lamTp", bufs=2))
        tmTp = ctx.enter_context(tc.tile_pool(name="tmTp", bufs=2))
        bTp = ctx.enter_context(tc.tile_pool(name="bTp", bufs=2))
        sTp = ctx.enter_context(tc.tile_pool(name="sTp", bufs=2))
        sgTp = ctx.enter_context(tc.tile_pool(name="sgTp", bufs=2))

        # ---- constants ----
        identf = const.tile([128, 128], f32, tag="identf", name="identf")
        masks.make_identity(nc, identf[:])
        identb = const.tile([128, 128], fr, tag="identb", name="identb")
        nc.gpsimd.tensor_copy(identb[:], identf[:])
        magic = const.tile([128, NTT], i32, tag="magic", name="magic")
        nc.gpsimd.memset(magic[:], MAGIC)

        nw = []
        for k in range(2):
            t = const.tile([128, 1], f32, tag=f"nw{k}", name=f"nw{k}")
            nc.sync.dma_start(t[:], nw_d[ts(k, 128)].rearrange("(a b) -> a b", b=1))
            nw.append(t)
        bdt2 = []
        for m in range(4):
            t = const.tile([128, 1], f32, tag=f"bdt{m}", name=f"bdt{m}")
            nc.sync.dma_start(t[:], bdt_d[ts(m, 128)].rearrange("(a b) -> a b", b=1))
            # scale in place: tanh(pre*0.5 + b_dt*0.5)
            nc.vector.tensor_scalar_mul(t[:], t[:], 0.5)
            bdt2.append(t)

        # ---- load + transpose weights (scoped pools: PSUM banks and the
        # raw-weight SBUF are released before the main loop) ----
        winT = [wT.tile([128, 1024], fr, tag=f"winT{k}", name=f"winT{k}")
                for k in range(2)]
        wdtT = [wT.tile([128, 512], fr, tag=f"wdtT{k}", name=f"wdtT{k}")
                for k in range(4)]
        woutT = [wT.tile([128, 256], fr, tag=f"woutT{e}", name=f"woutT{e}")
                 for e in range(4)]
        with tc.tile_pool(name="wraw", bufs=1) as wraw, \
                tc.tile_pool(name="wtps", bufs=2, space="PSUM") as wtps:
            # W_in [1024(feat), 256(d)] -> W_inT [2][128(d), 1024] * norm_w
            win_raw = []
            for f in range(8):
                t = wraw.tile([128, 256], f32, tag=f"winr{f}", name=f"winr{f}")
                nc.sync.dma_start(t[:], win_d[ts(f, 128), :])
                win_raw.append(t)
            for k in range(2):
                for half in range(2):
                    p = wtps.tile([128, 512], f32, tag="tpw", name="tpw")
                    for j in range(4):
                        f = half * 4 + j
                        nc.tensor.matmul(p[:, ts(j, 128)],
                                         win_raw[f][:, ts(k, 128)],
                                         identf[:], is_transpose=True)
                    if half % 2 == 0:
                        nc.vector.tensor_copy(winT[k][:, ts(half, 512)], p[:])
                    else:
                        nc.scalar.copy(winT[k][:, ts(half, 512)], p[:])
            for k in range(2):
                # fold norm_w (per-d row scale) into W_inT
                nc.vector.tensor_scalar_mul(winT[k][:], winT[k][:], nw[k][:])

            # W_dt [512(e_out), 512(e_in)] -> W_dtT [4][128(e_in), 512]
            wdt_raw = []
            for m in range(4):
                t = wraw.tile([128, 512], f32, tag=f"wdtr{m}", name=f"wdtr{m}")
                nc.sync.dma_start(t[:], wdt_d[ts(m, 128), :])
                wdt_raw.append(t)
            for k in range(4):
                p = wtps.tile([128, 512], f32, tag="tpw", name="tpw")
                for m in range(4):
                    nc.tensor.matmul(p[:, ts(m, 128)],
                                     wdt_raw[m][:, ts(k, 128)],
                                     identf[:], is_transpose=True)
                if k % 2 == 0:
                    nc.vector.tensor_copy(wdtT[k][:], p[:])
                else:
                    nc.scalar.copy(wdtT[k][:], p[:])

            # W_out [256(d), 512(e)] -> W_outT [4][128(e), 256(d)] * -0.5
            # (the -0.5 undoes the S = -2*s change of variable, so
            # sg = S*g can be a plain tensor mult)
            wout_raw = []
            for dd in range(2):
                t = wraw.tile([128, 512], f32, tag=f"woutr{dd}",
                              name=f"woutr{dd}")
                nc.sync.dma_start(t[:], wout_d[ts(dd, 128), :])
                wout_raw.append(t)
            for e in range(4):
                p = wtps.tile([128, 512], f32, tag="tpw", name="tpw")
                for dd in range(2):
                    nc.tensor.matmul(p[:, ts(dd, 128)],
                                     wout_raw[dd][:, ts(e, 128)],
                                     identf[:], is_transpose=True)
                nc.vector.tensor_scalar_mul(woutT[e][:], p[:, :256], -0.5)

        # main-loop PSUM pools, entered after the weight-stage PSUM pool
        # closes (8 banks total: 2+2+2+2)
        tps = ctx.enter_context(tc.tile_pool(name="tps", bufs=2, space="PSUM"))
        uvps = ctx.enter_context(tc.tile_pool(name="uvps", bufs=2, space="PSUM"))
        preps = ctx.enter_context(tc.tile_pool(name="preps", bufs=2, space="PSUM"))
        yps = ctx.enter_context(tc.tile_pool(name="yps", bufs=2, space="PSUM"))

        # src AP for x rows [c*512 .. c*512+512) as [128, t, d]
        def x_chunk_ap(c):
            lo = c * TC
            return x_d[lo:lo + TC, :].rearrange("(t p) d -> p t d", p=128)

        def out_pair_ap(c, pair):
            lo = c * TC + pair * 256
            return out_d[lo:lo + 256, :].rearrange("(t p) d -> p t d", p=128)

        # ---- main chunk loop ----
        def norm_stage(st, c):
            """Load x chunk, RMS stats (ACT), Newton-rsqrt (Pool),
            h = x*inv_rms in bf16 (Pool), PE-transpose -> hT."""
            xt = xp.tile([128, NTT * D], f32, tag="xt", name="xt")
            nc.sync.dma_start(
                xt[:].rearrange("p (t d) -> p t d", d=D), x_chunk_ap(c))

            # RMS stats: tensor_tensor_reduce hangs TRN2 HW, so split the
            # 4 subtile x^2-sums: 2 via ACT Square+accum, 2 via Pool
            # multiply + DVE free-axis reduce (balances all three engines)
            vpk = statp.tile([128, NTT], f32, tag="vpk", name="vpk")
            for t in range(2):
                sq = sqp.tile([128, D], f32, tag="sq", name="sq")
                nc.scalar.activation(sq[:], xt[:, ts(t, D)], AF.Square,
                                     accum_out=vpk[:, t:t + 1])
            for t in range(2, NTT):
                sq = sqp.tile([128, D], f32, tag="sq", name="sq")
                nc.gpsimd.tensor_mul(sq[:], xt[:, ts(t, D)], xt[:, ts(t, D)])
                nc.vector.tensor_reduce(vpk[:, t:t + 1], sq[:],
                                        axis=mybir.AxisListType.X,
                                        op=ALU.add)

            # inv_rms = rsqrt(vpk/D + eps) via Newton (nv + iters on Pool)
            nv = statp.tile([128, NTT], f32, tag="nv", name="nv")
            nc.gpsimd.tensor_scalar(nv[:], vpk[:], 1.0 / D, EPS,
                                    op0=ALU.mult, op1=ALU.add)
            ny = statp.tile([128, NTT], f32, tag="ny", name="ny")
            nt = statp.tile([128, NTT], f32, tag="nt", name="nt")
            # seed: y0 = bits(magic - (bits(v) >> 1)); MUST be the stt form
            # (a plain i32 tensor_sub runs through fp32 and loses low bits)
            nyi = ny[:].bitcast(i32)
            nc.vector.tensor_scalar(nyi, nv[:].bitcast(i32), 1, None,
                                    op0=ALU.arith_shift_right)
            nc.vector.scalar_tensor_tensor(nyi, magic[:], 1, nyi,
                                           op0=ALU.bypass, op1=ALU.subtract)
            for _ in range(2):
                # t = v*y*y ; y = y * (1.5 - 0.5*t)
                nc.gpsimd.tensor_mul(nt[:], ny[:], ny[:])
                nc.gpsimd.tensor_mul(nt[:], nt[:], nv[:])
                nc.gpsimd.tensor_scalar(nt[:], nt[:], -0.5, 1.5,
                                        op0=ALU.mult, op1=ALU.add)
                nc.gpsimd.tensor_mul(ny[:], ny[:], nt[:])

            # h = x * inv_rms (bf16, Pool); PE-transpose into ONE
            # [128,1024] bf16 PSUM bank; one wide DVE 2x copy -> hT
            hT = hTp.tile([128, 2 * TC], fr, tag="hT", name="hT")
            hs = []
            for t in range(NTT):
                h = hp.tile([128, D], fr, tag="h", name="h")
                nc.gpsimd.tensor_scalar_mul(h[:], xt[:, ts(t, D)],
                                            ny[:, t:t + 1])
                hs.append(h)
            p = tps.tile([128, 2 * TC], fr, tag="tp", name="tp")
            for k in range(2):
                for t in range(NTT):
                    nc.tensor.matmul(p[:, ts(k * NTT + t, 128)],
                                     hs[t][:, ts(k, 128)],
                                     identb[:] if mm_f32r else identf[:],
                                     is_transpose=True)
            nc.vector.tensor_copy(hT[:], p[:])
            st["hT"] = hT
            st["xt"] = xt

        def front_stage(st):
            """in_proj+silu, dt_proj+tanh, lam/tm (DVE), b (Pool)."""
            hT = st["hT"]
            uT = uTp.tile([128, WID], fr, tag="uT", name="uT")
            gT = gTp.tile([128, WID], fr, tag="gT", name="gT")
            for m in range(8):
                ps = uvps.tile([128, TC], f32, tag="uv", name="uv")
                for k in range(2):
                    nc.tensor.matmul(
                        ps[:], winT[k][:, ts(m, 128)], hT[:, ts(k, TC)],
                        start=(k == 0), stop=(k == 1),
                    )
                dst = (uT[:, ts(m, TC)] if m < 4
                       else gT[:, ts(m - 4, TC)])
                if silu_native:
                    nc.scalar.activation(dst, ps[:], AF.Silu)
                else:
                    # CoreSim has no Silu: decompose as x * sigmoid(x)
                    sig = sqp.tile([128, TC], f32, tag="sig", name="sig")
                    nc.scalar.activation(sig[:], ps[:], AF.Sigmoid)
                    nc.vector.tensor_mul(dst, ps[:], sig[:])

            thT = thTp.tile([128, WID], fr, tag="thT", name="thT")
            for m in range(4):
                ps = preps.tile([128, TC], f32, tag="pre", name="pre")
                for k in range(4):
                    nc.tensor.matmul(
                        ps[:], wdtT[k][:, ts(m, 128)], uT[:, ts(k, TC)],
                        start=(k == 0), stop=(k == 3),
                    )
                nc.scalar.activation(thT[:, ts(m, TC)], ps[:], AF.Tanh,
                                     bias=bdt2[m][:], scale=0.5)
            # lam = sigmoid(pre + b_dt) = 0.5*th + 0.5  (wide 4x-mode ts)
            lamT = lamTp.tile([128, WID], fr, tag="lamT", name="lamT")
            nc.vector.tensor_scalar(lamT[:], thT[:], 0.5, 0.5,
                                    op0=ALU.mult, op1=ALU.add)
            # tm = th - 1 (wide 4x-mode ts); b = tm*u (Pool, 2 halves)
            tmT = tmTp.tile([128, WID], fr, tag="tmT", name="tmT")
            nc.vector.tensor_scalar(tmT[:], thT[:], 1.0, None,
                                    op0=ALU.subtract)
            bT = bTp.tile([128, WID], fr, tag="bT", name="bT")
            for half in range(2):
                nc.gpsimd.tensor_mul(bT[:, ts(half, WID // 2)],
                                     tmT[:, ts(half, WID // 2)],
                                     uT[:, ts(half, WID // 2)])
            st.update(uT=uT, gT=gT, lamT=lamT, bT=bT)

        def scan_stage(st, sT_prev):
            # S_t = lam_t*S_{t-1} + b_t (S = -2*s); 4 DVE scans into
            # slices of a wide sT; then ONE wide 2x-mode sg = S*g.
            sT = sTp.tile([128, WID], fr, tag="sT", name="sT")
            sgT = sgTp.tile([128, WID], fr, tag="sgT", name="sgT")
            for m in range(4):
                init = (0.0 if sT_prev is None
                        else sT_prev[:, (m + 1) * TC - 1:(m + 1) * TC])
                nc.vector.tensor_tensor_scan(
                    sT[:, ts(m, TC)], st["lamT"][:, ts(m, TC)],
                    st["bT"][:, ts(m, TC)], init,
                    op0=ALU.mult, op1=ALU.add,
                )
            # sg = S*g (the -0.5 is folded into woutT)
            nc.vector.tensor_mul(sgT[:], sT[:], st["gT"][:])
            st.update(sT=sT, sgT=sgT)
            return sT

        def out_stage(st, c):
            # out_proj -> PSUM; fused residual-add drain (one wide stt
            # per half-chunk on DVE); one wide DMA out per half-chunk.
            sgT, xt = st["sgT"], st["xt"]
            for pair in range(2):
                yp = yps.tile([128, 512], f32, tag="y", name="y")
                tt = (2 * pair, 2 * pair + 1)
                for e in range(4):
                    for i, t in enumerate(tt):
                        # start=True zeroes the WHOLE 2KB psum bank, so
                        # only the first matmul into this tile sets it
                        nc.tensor.matmul(
                            yp[:, ts(i, D)], sgT[:, e * TC + t * 128:
                                                 e * TC + (t + 1) * 128],
                            woutT[e][:],
                            start=(e == 0 and i == 0), stop=(e == 3),
                            skip_group_check=True,
                        )
                ot = outp.tile([128, 512], f32, tag="ot", name="ot")
                nc.vector.scalar_tensor_tensor(
                    ot[:], yp[:], 0.0, xt[:, ts(pair, 512)],
                    op0=ALU.bypass, op1=ALU.add)
                nc.sync.dma_start(
                    out_pair_ap(c, pair),
                    ot[:].rearrange("p (t d) -> p t d", d=D))

        # Software-pipelined emission. Engines run their streams in order,
        # so next chunk's PE-heavy front must be emitted BEFORE this
        # chunk's out_proj for PE to stay busy during the scan tail.
        for _ in range(repeat):
            sts = [dict() for _ in range(NCHUNK)]
            norm_stage(sts[0], 0)
            front_stage(sts[0])
            if NCHUNK > 1:
                norm_stage(sts[1], 1)
            sT_prev = None
            for c in range(NCHUNK):
                sT_prev = scan_stage(sts[c], sT_prev)
                if c + 1 < NCHUNK:
                    front_stage(sts[c + 1])
                if c + 2 < NCHUNK:
                    norm_stage(sts[c + 2], c + 2)
                out_stage(sts[c], c)
                sts[c].clear()


def build(mm_f32r=True, silu_native=True, repeat=1):
    """Build and compile the Bass module (cached)."""
    key = ("nc", mm_f32r, silu_native, repeat)
    if key in _CACHE:
        return _CACHE[key]

    from concourse import bacc, mybir, tile

    f32 = mybir.dt.float32
    nc = bacc.Bacc(
        "TRN2",
        target_bir_lowering=False,
        debug=False,
        num_devices=N_CORES,
    )
    aps = {
        "x": nc.dram_tensor("x", [K, D], f32, kind="ExternalInput").ap(),
        "norm_w": nc.dram_tensor("norm_w", [D], f32, kind="ExternalInput").ap(),
        "W_in": nc.dram_tensor("W_in", [2 * INNER, D], f32, kind="ExternalInput").ap(),
        "W_dt": nc.dram_tensor("W_dt", [INNER, INNER], f32, kind="ExternalInput").ap(),
        "b_dt": nc.dram_tensor("b_dt", [INNER], f32, kind="ExternalInput").ap(),
        "W_out": nc.dram_tensor("W_out", [D, INNER], f32, kind="ExternalInput").ap(),
        "out": nc.dram_tensor("out", [K, D], f32, kind="ExternalOutput").ap(),
    }
    with tile.TileContext(nc) as tc:
        _emit(tc, aps, mm_f32r=mm_f32r, silu_native=silu_native,
              repeat=repeat)
    nc.compile()
    _CACHE[key] = nc
    return nc


def make_in_maps(inputs):
    x = np.asarray(inputs["x"], dtype=np.float32)
    shared = {
        "norm_w": np.asarray(inputs["norm_w"], dtype=np.float32),
        "W_in": np.asarray(inputs["W_in"], dtype=np.float32),
        "W_dt": np.asarray(inputs["W_dt"], dtype=np.float32),
        "b_dt": np.asarray(inputs["b_dt"], dtype=np.float32),
        "W_out": np.asarray(inputs["W_out"], dtype=np.float32),
    }
    return [
        {"x": np.ascontiguousarray(x[b]), **shared} for b in range(N_CORES)
    ]


def run(inputs, trace=False, mm_f32r=True, silu_native=True, **kw):
    from concourse.bass_utils import run_bass_kernel_spmd

    nc = build(mm_f32r=mm_f32r, silu_native=silu_native)
    in_maps = make_in_maps(inputs)
    res = run_bass_kernel_spmd(
        nc, in_maps, core_ids=list(range(N_CORES)), trace=trace, **kw
    )
    out = np.stack([res.results[b]["out"] for b in range(N_CORES)], axis=0)
    return out, res


def kernel(**inputs) -> np.ndarray:
    out, _ = run(inputs, trace=False)
    return out


# revision 21
# speedup vs baseline: 1.7138x; 1.1988x over previous
"""SlimMambaBlock Trainium2 kernel.

Full-input contract: kernel(**inputs) takes the complete tensors
(x [8, 4096, 256], norm_w [256], W_in [1024, 256], W_dt [512, 512],
b_dt [512], W_out [256, 512]) and returns the full output [8, 4096, 256].

Sharding: data-parallel over batch — core b processes batch b (8 cores).

Per-core program (Tile framework), feature-major activation layout,
1024-token chunks (4 chunks). Engine placement is ISA-constrained
(gpsimd has no scan / no 3-operand stt / no PSUM access on TRN2, and
tensor_tensor_reduce hangs TRN2 HW) and tuned so the serial DVE chain
(scan -> sg) is never extended by unrelated work:

  0. x is loaded ONCE per chunk directly as bf16 via gpsimd SWDGE
     cast-DMAs (DRAM f32 -> SBUF bf16).
  1. RMSNorm stats: x^2 subtile sums via Pool multiply + DVE free-axis
     reduce; inv_rms via Newton-rsqrt (bit-trick seed on DVE — the stt
     form, a plain i32 tensor_sub loses low bits through the fp32
     path — one iteration on Pool, packed [128,8]).
  2. h = x*inv_rms (Pool ts, bf16), PE-transpose (1 cyc/row) into
     [128,1024] bf16 PSUM banks; wide DVE copies (2x mode) -> hT.
  3. in_proj: uvT[feat, tok] = W_inT.T @ hT ; u = silu, g = silu (ACT)
     writing 512-token slices of wide [128,4096] bf16 tensors.
  4. dt_proj: preT = W_dtT.T @ uT ; th = tanh(pre/2 + b_dt/2) (ACT).
     lam = 0.5*th + 0.5 and tm = th - 1: one wide 4x-mode DVE ts each.
     b = tm*u: two [128,2048] in-place Pool tensor_muls.
  5. recurrence S_t = lam_t*S_{t-1} + b_t (S = -2*s) — 4 DVE scans
     (one per feature quarter) into slices of a wide sT.
  6. sg = S*g: ONE wide [128,4096] 2x-mode DVE tensor_mul (the -0.5
     that undoes the change of variable is pre-folded into W_outT).
  7. out_proj -> PSUM; the residual is accumulated by the PE itself
     (identity-stationary matmul over the bf16 x), so the PSUM->SBUF
     drain is a plain ACT Copy — the DVE never touches the out path.

Matmul operands are bf16 (PE 1 cycle/row vs 4 for fp32); the residual
add happens in bf16 (x rounded), well within the 2e-2 gate. Measured
rel err ~2e-3.

build(repeat=R) emits the main loop R times inside one NEFF; test.py
uses (time(R) - time(1))/(R-1) to cancel the large per-call dispatch
overhead of the axon tunnel when timing.
"""

import numpy as np

B, K, D = 8, 4096, 256
INNER = 512
EPS = 1e-5
TC = 1024                # tokens per chunk
NCHUNK = K // TC         # 4
NTT = TC // 128          # token-tiles per chunk (8)
WID = 4 * TC             # wide feature-major tensors: [128, 4*1024]
NEWTON_ITERS = 1

N_CORES = 8
MAGIC = 0x5F3759DF       # fast inverse sqrt seed

_CACHE: dict = {}


def _emit(tc, aps, mm_f32r=True, silu_native=True, repeat=1):
    """Emit the per-core program. aps: dict of DRAM APs."""
    import concourse.bass as bass
    import concourse.mybir as mybir
    from concourse import masks

    nc = tc.nc
    f32 = mybir.dt.float32
    i32 = mybir.dt.int32
    AF = mybir.ActivationFunctionType
    ALU = mybir.AluOpType
    ts = bass.ts

    fr = mybir.dt.bfloat16 if mm_f32r else f32

    x_d = aps["x"]
    nw_d = aps["norm_w"]
    win_d = aps["W_in"]
    wdt_d = aps["W_dt"]
    bdt_d = aps["b_dt"]
    wout_d = aps["W_out"]
    out_d = aps["out"]

    import contextlib
    ctx = contextlib.ExitStack()
    with ctx:
        const = ctx.enter_context(tc.tile_pool(name="const", bufs=1))
        wT = ctx.enter_context(tc.tile_pool(name="wT", bufs=1))
        xp = ctx.enter_context(tc.tile_pool(name="xp", bufs=3))
        outp = ctx.enter_context(tc.tile_pool(name="outp", bufs=4))
        sqp = ctx.enter_context(tc.tile_pool(name="sqp", bufs=2))
        statp = ctx.enter_context(tc.tile_pool(name="statp", bufs=3))
        hp = ctx.enter_context(tc.tile_pool(name="hp", bufs=2 * NTT))
        hTp = ctx.enter_context(tc.tile_pool(name="hTp", bufs=2))
        uTp = ctx.enter_context(tc.tile_pool(name="uTp", bufs=2))
        gTp = ctx.enter_context(tc.tile_pool(name="gTp", bufs=2))
        thTp = ctx.enter_context(tc.tile_pool(name="thTp", bufs=2))
        lamTp = ctx.enter_context(tc.tile_pool(name="lamTp", bufs=2))
        bTp = ctx.enter_context(tc.tile_pool(name="bTp", bufs=2))
        sTp = ctx.enter_context(tc.tile_pool(name="sTp", bufs=2))
        sgTp = ctx.enter_context(tc.tile_pool(name="sgTp", bufs=2))

        # ---- constants ----
        identf = const.tile([128, 128], f32, tag="identf", name="identf")
        masks.make_identity(nc, identf[:])
        identb = const.tile([128, 128], fr, tag="identb", name="identb")
        nc.gpsimd.tensor_copy(identb[:], identf[:])
        magic = const.tile([128, NTT], i32, tag="magic", name="magic")
        nc.gpsimd.memset(magic[:], MAGIC)

        def load_xb(c):
            """x chunk -> SBUF bf16 via two gpsimd SWDGE cast-DMAs."""
            xb = xp.tile([128, NTT * D], fr, tag="xb", name="xb")
            for half in range(2):
                lo = c * TC + half * 512
                src = x_d[lo:lo + 512, :].rearrange("(t p) d -> p t d", p=128)
                nc.gpsimd.dma_start(
                    xb[:, half * 4 * D:(half + 1) * 4 * D]
                    .rearrange("p (t d) -> p t d", d=D), src)
            return xb

        # prefetch the first two x chunks BEFORE the weight DMAs so the
        # compute-pipeline fill is not serialized behind them
        xbs = {c: load_xb(c) for c in range(min(2, NCHUNK))}

        nw = []
        for k in range(2):
            t = const.tile([128, 1], f32, tag=f"nw{k}", name=f"nw{k}")
            nc.sync.dma_start(t[:], nw_d[ts(k, 128)].rearrange("(a b) -> a b", b=1))
            nw.append(t)
        bdt2 = []
        for m in range(4):
            t = const.tile([128, 1], f32, tag=f"bdt{m}", name=f"bdt{m}")
            nc.sync.dma_start(t[:], bdt_d[ts(m, 128)].rearrange("(a b) -> a b", b=1))
            # scale in place: tanh(pre*0.5 + b_dt*0.5)
            nc.vector.tensor_scalar_mul(t[:], t[:], 0.5)
            bdt2.append(t)

        # ---- load + transpose weights (scoped pools; DMAs split across
        # the SP and ACT HWDGE queues to halve the serial issue time) ----
        winT = [wT.tile([128, 1024], fr, tag=f"winT{k}", name=f"winT{k}")
                for k in range(2)]
        wdtT = [wT.tile([128, 512], fr, tag=f"wdtT{k}", name=f"wdtT{k}")
                for k in range(4)]
        woutT = [wT.tile([128, 256], fr, tag=f"woutT{e}", name=f"woutT{e}")
                 for e in range(4)]
        with tc.tile_pool(name="wraw", bufs=1) as wraw, \
                tc.tile_pool(name="wtps", bufs=2, space="PSUM") as wtps:
            # W_in [1024(feat), 256(d)] -> W_inT [2][128(d), 1024] * norm_w
            win_raw = []
            for f in range(8):
                t = wraw.tile([128, 256], f32, tag=f"winr{f}", name=f"winr{f}")
                eng = nc.sync if f % 2 == 0 else nc.scalar
                eng.dma_start(t[:], win_d[ts(f, 128), :])
                win_raw.append(t)
            for k in range(2):
                for half in range(2):
                    p = wtps.tile([128, 512], f32, tag="tpw", name="tpw")
                    for j in range(4):
                        f = half * 4 + j
                        nc.tensor.matmul(p[:, ts(j, 128)],
                                         win_raw[f][:, ts(k, 128)],
                                         identf[:], is_transpose=True)
                    if half % 2 == 0:
                        nc.vector.tensor_copy(winT[k][:, ts(half, 512)], p[:])
                    else:
                        nc.scalar.copy(winT[k][:, ts(half, 512)], p[:])
            for k in range(2):
                # fold norm_w (per-d row scale) into W_inT
                nc.vector.tensor_scalar_mul(winT[k][:], winT[k][:], nw[k][:])

            # W_dt [512(e_out), 512(e_in)] -> W_dtT [4][128(e_in), 512]
            wdt_raw = []
            for m in range(4):
                t = wraw.tile([128, 512], f32, tag=f"wdtr{m}", name=f"wdtr{m}")
                eng = nc.sync if m % 2 == 0 else nc.scalar
                eng.dma_start(t[:], wdt_d[ts(m, 128), :])
                wdt_raw.append(t)
            for k in range(4):
                p = wtps.tile([128, 512], f32, tag="tpw", name="tpw")
                for m in range(4):
                    nc.tensor.matmul(p[:, ts(m, 128)],
                                     wdt_raw[m][:, ts(k, 128)],
                                     identf[:], is_transpose=True)
                if k % 2 == 0:
                    nc.vector.tensor_copy(wdtT[k][:], p[:])
                else:
                    nc.scalar.copy(wdtT[k][:], p[:])

            # W_out [256(d), 512(e)] -> W_outT [4][128(e), 256(d)] * -0.5
            # (the -0.5 undoes the S = -2*s change of variable, so
            # sg = S*g can be a plain tensor mult)
            wout_raw = []
            for dd in range(2):
                t = wraw.tile([128, 512], f32, tag=f"woutr{dd}",
                              name=f"woutr{dd}")
                eng = nc.sync if dd % 2 == 0 else nc.scalar
                eng.dma_start(t[:], wout_d[ts(dd, 128), :])
                wout_raw.append(t)
            for e in range(4):
                p = wtps.tile([128, 512], f32, tag="tpw", name="tpw")
                for dd in range(2):
                    nc.tensor.matmul(p[:, ts(dd, 128)],
                                     wout_raw[dd][:, ts(e, 128)],
                                     identf[:], is_transpose=True)
                nc.vector.tensor_scalar_mul(woutT[e][:], p[:, :256], -0.5)

        # main-loop PSUM pools, entered after the weight-stage PSUM pool
        # closes (8 banks total: 2+2+2+2)
        tps = ctx.enter_context(tc.tile_pool(name="tps", bufs=2, space="PSUM"))
        uvps = ctx.enter_context(tc.tile_pool(name="uvps", bufs=2, space="PSUM"))
        preps = ctx.enter_context(tc.tile_pool(name="preps", bufs=2, space="PSUM"))
        yps = ctx.enter_context(tc.tile_pool(name="yps", bufs=2, space="PSUM"))

        def out_pair_ap(c, pair):
            lo = c * TC + pair * 256
            return out_d[lo:lo + 256, :].rearrange("(t p) d -> p t d", p=128)

        # ---- main chunk loop ----
        def norm_stage(st, c):
            """Load x chunk (bf16), RMS stats (Pool+DVE), Newton-rsqrt,
            h = x*inv_rms (Pool), PE-transpose -> hT."""
            xb = xbs.pop(c) if c in xbs else load_xb(c)

            # x^2 subtile sums: Pool multiply + DVE free-axis reduce
            # (tensor_tensor_reduce hangs TRN2 HW; ACT is silu/tanh-bound)
            vpk = statp.tile([128, NTT], f32, tag="vpk", name="vpk")
            for t in range(NTT):
                sq = sqp.tile([128, D], f32, tag="sq", name="sq")
                nc.gpsimd.tensor_mul(sq[:], xb[:, ts(t, D)], xb[:, ts(t, D)])
                nc.vector.tensor_reduce(vpk[:, t:t + 1], sq[:],
                                        axis=mybir.AxisListType.X,
                                        op=ALU.add)

            # inv_rms = rsqrt(vpk/D + eps) via Newton (nv + iters on Pool)
            nv = statp.tile([128, NTT], f32, tag="nv", name="nv")
            nc.gpsimd.tensor_scalar(nv[:], vpk[:], 1.0 / D, EPS,
                                    op0=ALU.mult, op1=ALU.add)
            ny = statp.tile([128, NTT], f32, tag="ny", name="ny")
            nt = statp.tile([128, NTT], f32, tag="nt", name="nt")
            # seed: y0 = bits(magic - (bits(v) >> 1)); MUST be the stt form
            # (a plain i32 tensor_sub runs through fp32 and loses low bits)
            nyi = ny[:].bitcast(i32)
            nc.vector.tensor_scalar(nyi, nv[:].bitcast(i32), 1, None,
                                    op0=ALU.arith_shift_right)
            nc.vector.scalar_tensor_tensor(nyi, magic[:], 1, nyi,
                                           op0=ALU.bypass, op1=ALU.subtract)
            for _ in range(NEWTON_ITERS):
                # t = v*y*y ; y = y * (1.5 - 0.5*t)
                nc.gpsimd.tensor_mul(nt[:], ny[:], ny[:])
                nc.gpsimd.tensor_mul(nt[:], nt[:], nv[:])
                nc.gpsimd.tensor_scalar(nt[:], nt[:], -0.5, 1.5,
                                        op0=ALU.mult, op1=ALU.add)
                nc.gpsimd.tensor_mul(ny[:], ny[:], nt[:])

            # h = x * inv_rms (bf16, Pool); PE-transpose into [128,1024]
            # bf16 PSUM banks (one per d-half); wide DVE 2x copies -> hT
            hT = hTp.tile([128, 2 * TC], fr, tag="hT", name="hT")
            hs = []
            for t in range(NTT):
                h = hp.tile([128, D], fr, tag="h", name="h")
                nc.gpsimd.tensor_scalar_mul(h[:], xb[:, ts(t, D)],
                                            ny[:, t:t + 1])
                hs.append(h)
            for k in range(2):
                p = tps.tile([128, TC], fr, tag="tp", name="tp")
                for t in range(NTT):
                    nc.tensor.matmul(p[:, ts(t, 128)], hs[t][:, ts(k, 128)],
                                     identb[:] if mm_f32r else identf[:],
                                     is_transpose=True)
                nc.vector.tensor_copy(hT[:, ts(k, TC)], p[:])
            st["hT"] = hT
            st["xb"] = xb

        def front_stage(st):
            """in_proj+silu, dt_proj+tanh, lam/tm (DVE), b (Pool)."""
            hT = st["hT"]
            uT = uTp.tile([128, WID], fr, tag="uT", name="uT")
            gT = gTp.tile([128, WID], fr, tag="gT", name="gT")
            for m in range(8):
                for th2 in range(2):
                    ps = uvps.tile([128, 512], f32, tag="uv", name="uv")
                    for k in range(2):
                        nc.tensor.matmul(
                            ps[:], winT[k][:, ts(m, 128)],
                            hT[:, k * TC + th2 * 512:k * TC + (th2 + 1) * 512],
                            start=(k == 0), stop=(k == 1),
                        )
                    off = (m % 4) * TC + th2 * 512
                    dst = (uT[:, off:off + 512] if m < 4
                           else gT[:, off:off + 512])
                    if silu_native:
                        nc.scalar.activation(dst, ps[:], AF.Silu)
                    else:
                        # CoreSim has no Silu: decompose as x * sigmoid(x)
                        sig = sqp.tile([128, 512], f32, tag="sig", name="sig")
                        nc.scalar.activation(sig[:], ps[:], AF.Sigmoid)
                        nc.vector.tensor_mul(dst, ps[:], sig[:])

            thT = thTp.tile([128, WID], fr, tag="thT", name="thT")
            for m in range(4):
                for th2 in range(2):
                    ps = preps.tile([128, 512], f32, tag="pre", name="pre")
                    for k in range(4):
                        nc.tensor.matmul(
                            ps[:], wdtT[k][:, ts(m, 128)],
                            uT[:, k * TC + th2 * 512:k * TC + (th2 + 1) * 512],
                            start=(k == 0), stop=(k == 3),
                        )
                    nc.scalar.activation(
                        thT[:, m * TC + th2 * 512:m * TC + (th2 + 1) * 512],
                        ps[:], AF.Tanh, bias=bdt2[m][:], scale=0.5)
            # lam = sigmoid(pre + b_dt) = 0.5*th + 0.5  (wide 4x-mode ts)
            lamT = lamTp.tile([128, WID], fr, tag="lamT", name="lamT")
            nc.vector.tensor_scalar(lamT[:], thT[:], 0.5, 0.5,
                                    op0=ALU.mult, op1=ALU.add)
            # b = (th-1)*u: tm into bT (wide 4x-mode DVE ts), then an
            # in-place Pool multiply by u in two halves
            bT = bTp.tile([128, WID], fr, tag="bT", name="bT")
            nc.vector.tensor_scalar(bT[:], thT[:], 1.0, None,
                                    op0=ALU.subtract)
            for half in range(2):
                nc.gpsimd.tensor_mul(bT[:, ts(half, WID // 2)],
                                     bT[:, ts(half, WID // 2)],
                                     uT[:, ts(half, WID // 2)])
            st.update(uT=uT, gT=gT, lamT=lamT, bT=bT)

        def scan_stage(st, sT_prev):
            # S_t = lam_t*S_{t-1} + b_t (S = -2*s); 4 DVE scans into
            # slices of a wide sT; then ONE wide 2x-mode sg = S*g.
            sT = sTp.tile([128, WID], fr, tag="sT", name="sT")
            sgT = sgTp.tile([128, WID], fr, tag="sgT", name="sgT")
            for m in range(4):
                init = (0.0 if sT_prev is None
                        else sT_prev[:, (m + 1) * TC - 1:(m + 1) * TC])
                nc.vector.tensor_tensor_scan(
                    sT[:, ts(m, TC)], st["lamT"][:, ts(m, TC)],
                    st["bT"][:, ts(m, TC)], init,
                    op0=ALU.mult, op1=ALU.add,
                )
            # sg = S*g (the -0.5 is folded into woutT)
            nc.vector.tensor_mul(sgT[:], sT[:], st["gT"][:])
            st.update(sT=sT, sgT=sgT)
            return sT

        def out_stage(st, c):
            # out_proj -> PSUM, residual x added by the PE itself via an
            # identity-stationary matmul; plain ACT Copy drains; one wide
            # DMA out per 256 tokens.
            sgT, xb = st["sgT"], st["xb"]
            for pair in range(NTT // 2):
                yp = yps.tile([128, 512], f32, tag="y", name="y")
                tt = (2 * pair, 2 * pair + 1)
                for e in range(4):
                    for i, t in enumerate(tt):
                        # start=True zeroes the WHOLE 2KB psum bank, so
                        # only the first matmul into this tile sets it
                        nc.tensor.matmul(
                            yp[:, ts(i, D)], sgT[:, e * TC + t * 128:
                                                 e * TC + (t + 1) * 128],
                            woutT[e][:],
                            start=(e == 0 and i == 0), stop=False,
                            skip_group_check=True,
                        )
                for i, t in enumerate(tt):
                    nc.tensor.matmul(
                        yp[:, ts(i, D)], identb[:], xb[:, ts(t, D)],
                        start=False, stop=True, skip_group_check=True,
                    )
                ot = outp.tile([128, 512], f32, tag="ot", name="ot")
                nc.scalar.copy(ot[:], yp[:])
                nc.sync.dma_start(
                    out_pair_ap(c, pair),
                    ot[:].rearrange("p (t d) -> p t d", d=D))

        # Software-pipelined emission. Engines run their streams in order,
        # so next chunk's PE-heavy front must be emitted BEFORE this
        # chunk's out_proj for PE to stay busy during the scan tail.
        for _ in range(repeat):
            sts = [dict() for _ in range(NCHUNK)]
            norm_stage(sts[0], 0)
            front_stage(sts[0])
            if NCHUNK > 1:
                norm_stage(sts[1], 1)
            sT_prev = None
            for c in range(NCHUNK):
                sT_prev = scan_stage(sts[c], sT_prev)
                if c + 1 < NCHUNK:
                    front_stage(sts[c + 1])
                if c + 2 < NCHUNK:
                    norm_stage(sts[c + 2], c + 2)
                out_stage(sts[c], c)
                sts[c].clear()


def build(mm_f32r=True, silu_native=True, repeat=1):
    """Build and compile the Bass module (cached)."""
    key = ("nc", mm_f32r, silu_native, repeat)
    if key in _CACHE:
        return _CACHE[key]

    from concourse import bacc, mybir, tile

    f32 = mybir.dt.float32
    nc = bacc.Bacc(
        "TRN2",
        target_bir_lowering=False,
        debug=False,
        num_devices=N_CORES,
    )
    aps = {
        "x": nc.dram_tensor("x", [K, D], f32, kind="ExternalInput").ap(),
        "norm_w": nc.dram_tensor("norm_w", [D], f32, kind="ExternalInput").ap(),
        "W_in": nc.dram_tensor("W_in", [2 * INNER, D], f32, kind="ExternalInput").ap(),
        "W_dt": nc.dram_tensor("W_dt", [INNER, INNER], f32, kind="ExternalInput").ap(),
        "b_dt": nc.dram_tensor("b_dt", [INNER], f32, kind="ExternalInput").ap(),
        "W_out": nc.dram_tensor("W_out", [D, INNER], f32, kind="ExternalInput").ap(),
        "out": nc.dram_tensor("out", [K, D], f32, kind="ExternalOutput").ap(),
    }
    with tile.TileContext(nc) as tc:
        _emit(tc, aps, mm_f32r=mm_f32r, silu_native=silu_native,
              repeat=repeat)
    nc.compile()
    _CACHE[key] = nc
    return nc


def make_in_maps(inputs):
    x = np.asarray(inputs["x"], dtype=np.float32)
    shared = {
        "norm_w": np.asarray(inputs["norm_w"], dtype=np.float32),
        "W_in": np.asarray(inputs["W_in"], dtype=np.float32),
        "W_dt": np.asarray(inputs["W_dt"], dtype=np.float32),
        "b_dt": np.asarray(inputs["b_dt"], dtype=np.float32),
        "W_out": np.asarray(inputs["W_out"], dtype=np.float32),
    }
    return [
        {"x": np.ascontiguousarray(x[b]), **shared} for b in range(N_CORES)
    ]


def run(inputs, trace=False, mm_f32r=True, silu_native=True, **kw):
    from concourse.bass_utils import run_bass_kernel_spmd

    nc = build(mm_f32r=mm_f32r, silu_native=silu_native)
    in_maps = make_in_maps(inputs)
    res = run_bass_kernel_spmd(
        nc, in_maps, core_ids=list(range(N_CORES)), trace=trace, **kw
    )
    out = np.stack([res.results[b]["out"] for b in range(N_CORES)], axis=0)
    return out, res


def kernel(**inputs) -> np.ndarray:
    out, _ = run(inputs, trace=False)
    return out


# revision 24
# speedup vs baseline: 1.7792x; 1.0381x over previous
"""SlimMambaBlock Trainium2 kernel.

Full-input contract: kernel(**inputs) takes the complete tensors
(x [8, 4096, 256], norm_w [256], W_in [1024, 256], W_dt [512, 512],
b_dt [512], W_out [256, 512]) and returns the full output [8, 4096, 256].

Sharding: data-parallel over batch — core b processes batch b (8 cores).

Per-core program (Tile framework), feature-major activation layout,
1024-token chunks (4 chunks). Engine placement is ISA-constrained
(gpsimd has no scan / no 3-operand stt / no PSUM access on TRN2, and
tensor_tensor_reduce hangs TRN2 HW) and tuned so the serial DVE chain
(scan -> sg) is never extended by unrelated work:

  0. x is loaded ONCE per chunk directly as bf16 via gpsimd SWDGE
     cast-DMAs (DRAM f32 -> SBUF bf16).
  1. RMSNorm stats: x^2 subtile sums via Pool multiply + DVE free-axis
     reduce; inv_rms via Newton-rsqrt (bit-trick seed on DVE — the stt
     form, a plain i32 tensor_sub loses low bits through the fp32
     path — one iteration on Pool, packed [128,8]).
  2. h = x*inv_rms (Pool ts, bf16), PE-transpose (1 cyc/row) into
     [128,1024] bf16 PSUM banks; wide DVE copies (2x mode) -> hT.
  3. in_proj: uvT[feat, tok] = W_inT.T @ hT ; u = silu, g = silu (ACT)
     writing 512-token slices of wide [128,4096] bf16 tensors.
  4. dt_proj: preT = W_dtT.T @ uT ; th = tanh(pre/2 + b_dt/2) (ACT).
     lam = 0.5*th + 0.5 and tm = th - 1: one wide 4x-mode DVE ts each.
     b = tm*u: two [128,2048] in-place Pool tensor_muls.
  5. recurrence S_t = lam_t*S_{t-1} + b_t (S = -2*s) — 4 DVE scans
     (one per feature quarter) into slices of a wide sT.
  6. sg = S*g: ONE wide [128,4096] 2x-mode DVE tensor_mul (the -0.5
     that undoes the change of variable is pre-folded into W_outT).
  7. out_proj -> PSUM; the residual is accumulated by the PE itself
     (identity-stationary matmul over the bf16 x), so the PSUM->SBUF
     drain is a plain ACT Copy — the DVE never touches the out path.

Matmul operands are bf16 (PE 1 cycle/row vs 4 for fp32); the residual
add happens in bf16 (x rounded), well within the 2e-2 gate. Measured
rel err ~2e-3.

build(repeat=R) emits the main loop R times inside one NEFF; test.py
uses (time(R) - time(1))/(R-1) to cancel the large per-call dispatch
overhead of the axon tunnel when timing.
"""

import numpy as np

B, K, D = 8, 4096, 256
INNER = 512
EPS = 1e-5
TC = 1024                # tokens per chunk
NCHUNK = K // TC         # 4
NTT = TC // 128          # token-tiles per chunk (8)
WID = 4 * TC             # wide feature-major tensors: [128, 4*1024]
NEWTON_ITERS = 1

N_CORES = 8
MAGIC = 0x5F3759DF       # fast inverse sqrt seed

_CACHE: dict = {}


def _emit(tc, aps, mm_f32r=True, silu_native=True, repeat=1):
    """Emit the per-core program. aps: dict of DRAM APs."""
    import concourse.bass as bass
    import concourse.mybir as mybir
    from concourse import masks

    nc = tc.nc
    f32 = mybir.dt.float32
    i32 = mybir.dt.int32
    AF = mybir.ActivationFunctionType
    ALU = mybir.AluOpType
    ts = bass.ts

    fr = mybir.dt.bfloat16 if mm_f32r else f32

    x_d = aps["x"]
    nw_d = aps["norm_w"]
    win_d = aps["W_in"]
    wdt_d = aps["W_dt"]
    bdt_d = aps["b_dt"]
    wout_d = aps["W_out"]
    out_d = aps["out"]

    import contextlib
    ctx = contextlib.ExitStack()
    with ctx:
        const = ctx.enter_context(tc.tile_pool(name="const", bufs=1))
        wT = ctx.enter_context(tc.tile_pool(name="wT", bufs=1))
        xp = ctx.enter_context(tc.tile_pool(name="xp", bufs=3))
        outp = ctx.enter_context(tc.tile_pool(name="outp", bufs=4))
        sqp = ctx.enter_context(tc.tile_pool(name="sqp", bufs=2))
        statp = ctx.enter_context(tc.tile_pool(name="statp", bufs=3))
        hp = ctx.enter_context(tc.tile_pool(name="hp", bufs=2 * NTT))
        hTp = ctx.enter_context(tc.tile_pool(name="hTp", bufs=2))
        uTp = ctx.enter_context(tc.tile_pool(name="uTp", bufs=2))
        gTp = ctx.enter_context(tc.tile_pool(name="gTp", bufs=2))
        thTp = ctx.enter_context(tc.tile_pool(name="thTp", bufs=2))
        lamTp = ctx.enter_context(tc.tile_pool(name="lamTp", bufs=2))
        bTp = ctx.enter_context(tc.tile_pool(name="bTp", bufs=2))
        sTp = ctx.enter_context(tc.tile_pool(name="sTp", bufs=2))
        sgTp = ctx.enter_context(tc.tile_pool(name="sgTp", bufs=2))

        # ---- constants ----
        identf = const.tile([128, 128], f32, tag="identf", name="identf")
        masks.make_identity(nc, identf[:])
        identb = const.tile([128, 128], fr, tag="identb", name="identb")
        nc.gpsimd.tensor_copy(identb[:], identf[:])
        magic = const.tile([128, NTT], i32, tag="magic", name="magic")
        nc.gpsimd.memset(magic[:], MAGIC)

        def load_xb(c):
            """x chunk -> SBUF bf16 via two gpsimd SWDGE cast-DMAs."""
            xb = xp.tile([128, NTT * D], fr, tag="xb", name="xb")
            for half in range(2):
                lo = c * TC + half * 512
                src = x_d[lo:lo + 512, :].rearrange("(t p) d -> p t d", p=128)
                nc.gpsimd.dma_start(
                    xb[:, half * 4 * D:(half + 1) * 4 * D]
                    .rearrange("p (t d) -> p t d", d=D), src)
            return xb

        # prefetch the first two x chunks BEFORE the weight DMAs so the
        # compute-pipeline fill is not serialized behind them
        xbs = {c: load_xb(c) for c in range(min(2, NCHUNK))}

        nw = []
        for k in range(2):
            t = const.tile([128, 1], f32, tag=f"nw{k}", name=f"nw{k}")
            nc.sync.dma_start(t[:], nw_d[ts(k, 128)].rearrange("(a b) -> a b", b=1))
            nw.append(t)
        bdt2 = []
        for m in range(4):
            t = const.tile([128, 1], f32, tag=f"bdt{m}", name=f"bdt{m}")
            nc.sync.dma_start(t[:], bdt_d[ts(m, 128)].rearrange("(a b) -> a b", b=1))
            # scale in place: tanh(pre*0.5 + b_dt*0.5)
            nc.vector.tensor_scalar_mul(t[:], t[:], 0.5)
            bdt2.append(t)

        # ---- load + transpose weights (scoped pools; DMAs split across
        # the SP and ACT HWDGE queues to halve the serial issue time) ----
        winT = [wT.tile([128, 1024], fr, tag=f"winT{k}", name=f"winT{k}")
                for k in range(2)]
        wdtT = [wT.tile([128, 512], fr, tag=f"wdtT{k}", name=f"wdtT{k}")
                for k in range(4)]
        woutT = [wT.tile([128, 256], fr, tag=f"woutT{e}", name=f"woutT{e}")
                 for e in range(4)]
        with tc.tile_pool(name="wraw", bufs=1) as wraw, \
                tc.tile_pool(name="wtps", bufs=2, space="PSUM") as wtps:
            # W_in [1024(feat), 256(d)] -> W_inT [2][128(d), 1024] * norm_w
            win_raw = []
            for f in range(8):
                t = wraw.tile([128, 256], f32, tag=f"winr{f}", name=f"winr{f}")
                eng = nc.sync if f % 2 == 0 else nc.scalar
                eng.dma_start(t[:], win_d[ts(f, 128), :])
                win_raw.append(t)
            for k in range(2):
                for half in range(2):
                    p = wtps.tile([128, 512], f32, tag="tpw", name="tpw")
                    for j in range(4):
                        f = half * 4 + j
                        nc.tensor.matmul(p[:, ts(j, 128)],
                                         win_raw[f][:, ts(k, 128)],
                                         identf[:], is_transpose=True)
                    if half % 2 == 0:
                        nc.vector.tensor_copy(winT[k][:, ts(half, 512)], p[:])
                    else:
                        nc.scalar.copy(winT[k][:, ts(half, 512)], p[:])
            for k in range(2):
                # fold norm_w (per-d row scale) into W_inT
                nc.vector.tensor_scalar_mul(winT[k][:], winT[k][:], nw[k][:])

            # W_dt [512(e_out), 512(e_in)] -> W_dtT [4][128(e_in), 512]
            wdt_raw = []
            for m in range(4):
                t = wraw.tile([128, 512], f32, tag=f"wdtr{m}", name=f"wdtr{m}")
                eng = nc.sync if m % 2 == 0 else nc.scalar
                eng.dma_start(t[:], wdt_d[ts(m, 128), :])
                wdt_raw.append(t)
            for k in range(4):
                p = wtps.tile([128, 512], f32, tag="tpw", name="tpw")
                for m in range(4):
                    nc.tensor.matmul(p[:, ts(m, 128)],
                                     wdt_raw[m][:, ts(k, 128)],
                                     identf[:], is_transpose=True)
                if k % 2 == 0:
                    nc.vector.tensor_copy(wdtT[k][:], p[:])
                else:
                    nc.scalar.copy(wdtT[k][:], p[:])

            # W_out [256(d), 512(e)] -> W_outT [4][128(e), 256(d)] * -0.5
            # (the -0.5 undoes the S = -2*s change of variable, so
            # sg = S*g can be a plain tensor mult)
            wout_raw = []
            for dd in range(2):
                t = wraw.tile([128, 512], f32, tag=f"woutr{dd}",
                              name=f"woutr{dd}")
                eng = nc.sync if dd % 2 == 0 else nc.scalar
                eng.dma_start(t[:], wout_d[ts(dd, 128), :])
                wout_raw.append(t)
            for e in range(4):
                p = wtps.tile([128, 512], f32, tag="tpw", name="tpw")
                for dd in range(2):
                    nc.tensor.matmul(p[:, ts(dd, 128)],
                                     wout_raw[dd][:, ts(e, 128)],
                                     identf[:], is_transpose=True)
                nc.vector.tensor_scalar_mul(woutT[e][:], p[:, :256], -0.5)

        # main-loop PSUM pools, entered after the weight-stage PSUM pool
        # closes (8 banks total: 2+2+2+2)
        tps = ctx.enter_context(tc.tile_pool(name="tps", bufs=2, space="PSUM"))
        uvps = ctx.enter_context(tc.tile_pool(name="uvps", bufs=2, space="PSUM"))
        preps = ctx.enter_context(tc.tile_pool(name="preps", bufs=2, space="PSUM"))
        yps = ctx.enter_context(tc.tile_pool(name="yps", bufs=2, space="PSUM"))

        def out_pair_ap(c, pair):
            lo = c * TC + pair * 256
            return out_d[lo:lo + 256, :].rearrange("(t p) d -> p t d", p=128)

        # ---- main chunk loop ----
        def norm_stage(st, c):
            """Load x chunk (bf16), RMS stats (Pool+DVE), Newton-rsqrt,
            h = x*inv_rms (Pool), PE-transpose -> hT."""
            xb = xbs.pop(c) if c in xbs else load_xb(c)

            # x^2 subtile sums: Pool multiply + DVE free-axis reduce
            # (tensor_tensor_reduce hangs TRN2 HW; ACT is silu/tanh-bound)
            vpk = statp.tile([128, NTT], f32, tag="vpk", name="vpk")
            for t in range(NTT):
                sq = sqp.tile([128, D], f32, tag="sq", name="sq")
                nc.gpsimd.tensor_mul(sq[:], xb[:, ts(t, D)], xb[:, ts(t, D)])
                nc.vector.tensor_reduce(vpk[:, t:t + 1], sq[:],
                                        axis=mybir.AxisListType.X,
                                        op=ALU.add)

            # inv_rms = rsqrt(vpk/D + eps) via Newton (nv + iters on Pool)
            nv = statp.tile([128, NTT], f32, tag="nv", name="nv")
            nc.gpsimd.tensor_scalar(nv[:], vpk[:], 1.0 / D, EPS,
                                    op0=ALU.mult, op1=ALU.add)
            ny = statp.tile([128, NTT], f32, tag="ny", name="ny")
            nt = statp.tile([128, NTT], f32, tag="nt", name="nt")
            # seed: y0 = bits(magic - (bits(v) >> 1)); MUST be the stt form
            # (a plain i32 tensor_sub runs through fp32 and loses low bits)
            nyi = ny[:].bitcast(i32)
            nc.vector.tensor_scalar(nyi, nv[:].bitcast(i32), 1, None,
                                    op0=ALU.arith_shift_right)
            nc.vector.scalar_tensor_tensor(nyi, magic[:], 1, nyi,
                                           op0=ALU.bypass, op1=ALU.subtract)
            for _ in range(NEWTON_ITERS):
                # t = v*y*y ; y = y * (1.5 - 0.5*t)
                nc.gpsimd.tensor_mul(nt[:], ny[:], ny[:])
                nc.gpsimd.tensor_mul(nt[:], nt[:], nv[:])
                nc.gpsimd.tensor_scalar(nt[:], nt[:], -0.5, 1.5,
                                        op0=ALU.mult, op1=ALU.add)
                nc.gpsimd.tensor_mul(ny[:], ny[:], nt[:])

            # h = x * inv_rms (bf16, Pool); PE-transpose into [128,1024]
            # bf16 PSUM banks (one per d-half); wide DVE 2x copies -> hT
            hT = hTp.tile([128, 2 * TC], fr, tag="hT", name="hT")
            hs = []
            for t in range(NTT):
                h = hp.tile([128, D], fr, tag="h", name="h")
                nc.gpsimd.tensor_scalar_mul(h[:], xb[:, ts(t, D)],
                                            ny[:, t:t + 1])
                hs.append(h)
            for k in range(2):
                p = tps.tile([128, TC], fr, tag="tp", name="tp")
                for t in range(NTT):
                    nc.tensor.matmul(p[:, ts(t, 128)], hs[t][:, ts(k, 128)],
                                     identb[:] if mm_f32r else identf[:],
                                     is_transpose=True)
                nc.vector.tensor_copy(hT[:, ts(k, TC)], p[:])
            st["hT"] = hT
            st["xb"] = xb

        def front_stage(st):
            """in_proj+silu, dt_proj+tanh, lam/tm (DVE), b (Pool)."""
            hT = st["hT"]
            uT = uTp.tile([128, WID], fr, tag="uT", name="uT")
            gT = gTp.tile([128, WID], fr, tag="gT", name="gT")
            for m in range(8):
                for th2 in range(2):
                    ps = uvps.tile([128, 512], f32, tag="uv", name="uv")
                    for k in range(2):
                        nc.tensor.matmul(
                            ps[:], winT[k][:, ts(m, 128)],
                            hT[:, k * TC + th2 * 512:k * TC + (th2 + 1) * 512],
                            start=(k == 0), stop=(k == 1),
                        )
                    off = (m % 4) * TC + th2 * 512
                    dst = (uT[:, off:off + 512] if m < 4
                           else gT[:, off:off + 512])
                    if silu_native:
                        nc.scalar.activation(dst, ps[:], AF.Silu)
                    else:
                        # CoreSim has no Silu: decompose as x * sigmoid(x)
                        sig = sqp.tile([128, 512], f32, tag="sig", name="sig")
                        nc.scalar.activation(sig[:], ps[:], AF.Sigmoid)
                        nc.vector.tensor_mul(dst, ps[:], sig[:])

            thT = thTp.tile([128, WID], fr, tag="thT", name="thT")
            for m in range(4):
                for th2 in range(2):
                    ps = preps.tile([128, 512], f32, tag="pre", name="pre")
                    for k in range(4):
                        nc.tensor.matmul(
                            ps[:], wdtT[k][:, ts(m, 128)],
                            uT[:, k * TC + th2 * 512:k * TC + (th2 + 1) * 512],
                            start=(k == 0), stop=(k == 3),
                        )
                    nc.scalar.activation(
                        thT[:, m * TC + th2 * 512:m * TC + (th2 + 1) * 512],
                        ps[:], AF.Tanh, bias=bdt2[m][:], scale=0.5)
            # lam = sigmoid(pre + b_dt) = 0.5*th + 0.5 and b = (th-1)*u,
            # per feature-quarter (4x-mode DVE ts + in-place Pool mult)
            # so scan(m) can start after 1/4 of the b-work, shortening
            # the serial tail of the last chunk
            lamT = lamTp.tile([128, WID], fr, tag="lamT", name="lamT")
            bT = bTp.tile([128, WID], fr, tag="bT", name="bT")
            for m in range(4):
                nc.vector.tensor_scalar(bT[:, ts(m, TC)], thT[:, ts(m, TC)],
                                        1.0, None, op0=ALU.subtract)
                nc.vector.tensor_scalar(lamT[:, ts(m, TC)],
                                        thT[:, ts(m, TC)], 0.5, 0.5,
                                        op0=ALU.mult, op1=ALU.add)
                nc.gpsimd.tensor_mul(bT[:, ts(m, TC)], bT[:, ts(m, TC)],
                                     uT[:, ts(m, TC)])
            st.update(uT=uT, gT=gT, lamT=lamT, bT=bT)

        def scan_stage(st, sT_prev):
            # S_t = lam_t*S_{t-1} + b_t (S = -2*s); 4 DVE scans into
            # slices of a wide sT; then ONE wide 2x-mode sg = S*g.
            sT = sTp.tile([128, WID], fr, tag="sT", name="sT")
            sgT = sgTp.tile([128, WID], fr, tag="sgT", name="sgT")
            for m in range(4):
                init = (0.0 if sT_prev is None
                        else sT_prev[:, (m + 1) * TC - 1:(m + 1) * TC])
                nc.vector.tensor_tensor_scan(
                    sT[:, ts(m, TC)], st["lamT"][:, ts(m, TC)],
                    st["bT"][:, ts(m, TC)], init,
                    op0=ALU.mult, op1=ALU.add,
                )
            # sg = S*g (the -0.5 is folded into woutT)
            nc.vector.tensor_mul(sgT[:], sT[:], st["gT"][:])
            st.update(sT=sT, sgT=sgT)
            return sT

        def out_stage(st, c):
            # out_proj -> PSUM, residual x added by the PE itself via an
            # identity-stationary matmul; plain ACT Copy drains; one wide
            # DMA out per 256 tokens.
            sgT, xb = st["sgT"], st["xb"]
            for pair in range(NTT // 2):
                yp = yps.tile([128, 512], f32, tag="y", name="y")
                tt = (2 * pair, 2 * pair + 1)
                for e in range(4):
                    for i, t in enumerate(tt):
                        # start=True zeroes the WHOLE 2KB psum bank, so
                        # only the first matmul into this tile sets it
                        nc.tensor.matmul(
                            yp[:, ts(i, D)], sgT[:, e * TC + t * 128:
                                                 e * TC + (t + 1) * 128],
                            woutT[e][:],
                            start=(e == 0 and i == 0), stop=False,
                            skip_group_check=True,
                        )
                for i, t in enumerate(tt):
                    nc.tensor.matmul(
                        yp[:, ts(i, D)], identb[:], xb[:, ts(t, D)],
                        start=False, stop=True, skip_group_check=True,
                    )
                ot = outp.tile([128, 512], f32, tag="ot", name="ot")
                nc.scalar.copy(ot[:], yp[:])
                nc.sync.dma_start(
                    out_pair_ap(c, pair),
                    ot[:].rearrange("p (t d) -> p t d", d=D))

        # Software-pipelined emission. Engines run their streams in order,
        # so next chunk's PE-heavy front must be emitted BEFORE this
        # chunk's out_proj for PE to stay busy during the scan tail.
        for _ in range(repeat):
            sts = [dict() for _ in range(NCHUNK)]
            norm_stage(sts[0], 0)
            front_stage(sts[0])
            if NCHUNK > 1:
                norm_stage(sts[1], 1)
            sT_prev = None
            for c in range(NCHUNK):
                sT_prev = scan_stage(sts[c], sT_prev)
                if c + 1 < NCHUNK:
                    front_stage(sts[c + 1])
                if c + 2 < NCHUNK:
                    norm_stage(sts[c + 2], c + 2)
                out_stage(sts[c], c)
                sts[c].clear()


def build(mm_f32r=True, silu_native=True, repeat=1):
    """Build and compile the Bass module (cached)."""
    key = ("nc", mm_f32r, silu_native, repeat)
    if key in _CACHE:
        return _CACHE[key]

    from concourse import bacc, mybir, tile

    f32 = mybir.dt.float32
    nc = bacc.Bacc(
        "TRN2",
        target_bir_lowering=False,
        debug=False,
        num_devices=N_CORES,
    )
    aps = {
        "x": nc.dram_tensor("x", [K, D], f32, kind="ExternalInput").ap(),
        "norm_w": nc.dram_tensor("norm_w", [D], f32, kind="ExternalInput").ap(),
        "W_in": nc.dram_tensor("W_in", [2 * INNER, D], f32, kind="ExternalInput").ap(),
        "W_dt": nc.dram_tensor("W_dt", [INNER, INNER], f32, kind="ExternalInput").ap(),
        "b_dt": nc.dram_tensor("b_dt", [INNER], f32, kind="ExternalInput").ap(),
        "W_out": nc.dram_tensor("W_out", [D, INNER], f32, kind="ExternalInput").ap(),
        "out": nc.dram_tensor("out", [K, D], f32, kind="ExternalOutput").ap(),
    }
    with tile.TileContext(nc) as tc:
        _emit(tc, aps, mm_f32r=mm_f32r, silu_native=silu_native,
              repeat=repeat)
    nc.compile()
    _CACHE[key] = nc
    return nc


def make_in_maps(inputs):
    x = np.asarray(inputs["x"], dtype=np.float32)
    shared = {
        "norm_w": np.asarray(inputs["norm_w"], dtype=np.float32),
        "W_in": np.asarray(inputs["W_in"], dtype=np.float32),
        "W_dt": np.asarray(inputs["W_dt"], dtype=np.float32),
        "b_dt": np.asarray(inputs["b_dt"], dtype=np.float32),
        "W_out": np.asarray(inputs["W_out"], dtype=np.float32),
    }
    return [
        {"x": np.ascontiguousarray(x[b]), **shared} for b in range(N_CORES)
    ]


def run(inputs, trace=False, mm_f32r=True, silu_native=True, **kw):
    from concourse.bass_utils import run_bass_kernel_spmd

    nc = build(mm_f32r=mm_f32r, silu_native=silu_native)
    in_maps = make_in_maps(inputs)
    res = run_bass_kernel_spmd(
        nc, in_maps, core_ids=list(range(N_CORES)), trace=trace, **kw
    )
    out = np.stack([res.results[b]["out"] for b in range(N_CORES)], axis=0)
    return out, res


def kernel(**inputs) -> np.ndarray:
    out, _ = run(inputs, trace=False)
    return out


# revision 27
# speedup vs baseline: 1.8639x; 1.0476x over previous
"""SlimMambaBlock Trainium2 kernel.

Full-input contract: kernel(**inputs) takes the complete tensors
(x [8, 4096, 256], norm_w [256], W_in [1024, 256], W_dt [512, 512],
b_dt [512], W_out [256, 512]) and returns the full output [8, 4096, 256].

Sharding: data-parallel over batch — core b processes batch b (8 cores).

Per-core program (Tile framework), feature-major activation layout,
1024-token chunks (4 chunks). Engine placement is ISA-constrained
(gpsimd has no scan / no 3-operand stt / no PSUM access on TRN2, and
tensor_tensor_reduce hangs TRN2 HW) and tuned so the serial DVE chain
(scan -> sg) is never extended by unrelated work:

  0. x is loaded ONCE per chunk directly as bf16 via gpsimd SWDGE
     cast-DMAs (DRAM f32 -> SBUF bf16).
  1. RMSNorm stats: x^2 subtile sums via Pool multiply + DVE free-axis
     reduce; inv_rms via Newton-rsqrt (bit-trick seed on DVE — the stt
     form, a plain i32 tensor_sub loses low bits through the fp32
     path — one iteration on Pool, packed [128,8]).
  2. h = x*inv_rms (Pool ts, bf16), PE-transpose (1 cyc/row) into
     [128,1024] bf16 PSUM banks; wide DVE copies (2x mode) -> hT.
  3. in_proj: uvT[feat, tok] = W_inT.T @ hT ; u = silu, g = silu (ACT)
     writing 512-token slices of wide [128,4096] bf16 tensors.
  4. dt_proj: preT = W_dtT.T @ uT ; th = tanh(pre/2 + b_dt/2) (ACT).
     lam = 0.5*th + 0.5 and tm = th - 1: one wide 4x-mode DVE ts each.
     b = tm*u: two [128,2048] in-place Pool tensor_muls.
  5. recurrence S_t = lam_t*S_{t-1} + b_t (S = -2*s) — 4 DVE scans
     (one per feature quarter) into slices of a wide sT.
  6. sg = S*g: ONE wide [128,4096] 2x-mode DVE tensor_mul (the -0.5
     that undoes the change of variable is pre-folded into W_outT).
  7. out_proj -> PSUM; the residual is accumulated by the PE itself
     (identity-stationary matmul over the bf16 x), so the PSUM->SBUF
     drain is a plain ACT Copy — the DVE never touches the out path.

Matmul operands are bf16 (PE 1 cycle/row vs 4 for fp32); the residual
add happens in bf16 (x rounded), well within the 2e-2 gate. Measured
rel err ~2e-3.

build(repeat=R) emits the main loop R times inside one NEFF; test.py
uses (time(R) - time(1))/(R-1) to cancel the large per-call dispatch
overhead of the axon tunnel when timing.
"""

import numpy as np

B, K, D = 8, 4096, 256
INNER = 512
EPS = 1e-5
TC = 1024                # tokens per chunk
NCHUNK = K // TC         # 4
NTT = TC // 128          # token-tiles per chunk (8)
WID = 4 * TC             # wide feature-major tensors: [128, 4*1024]
NEWTON_ITERS = 1

N_CORES = 8
MAGIC = 0x5F3759DF       # fast inverse sqrt seed

_CACHE: dict = {}


def _emit(tc, aps, mm_f32r=True, silu_native=True, repeat=1):
    """Emit the per-core program. aps: dict of DRAM APs."""
    import concourse.bass as bass
    import concourse.mybir as mybir
    from concourse import masks

    nc = tc.nc
    f32 = mybir.dt.float32
    i32 = mybir.dt.int32
    AF = mybir.ActivationFunctionType
    ALU = mybir.AluOpType
    ts = bass.ts

    fr = mybir.dt.bfloat16 if mm_f32r else f32

    x_d = aps["x"]
    nw_d = aps["norm_w"]
    win_d = aps["W_in"]
    wdt_d = aps["W_dt"]
    bdt_d = aps["b_dt"]
    wout_d = aps["W_out"]
    out_d = aps["out"]

    import contextlib
    ctx = contextlib.ExitStack()
    with ctx:
        const = ctx.enter_context(tc.tile_pool(name="const", bufs=1))
        wT = ctx.enter_context(tc.tile_pool(name="wT", bufs=1))
        xp = ctx.enter_context(tc.tile_pool(name="xp", bufs=3))
        outp = ctx.enter_context(tc.tile_pool(name="outp", bufs=4))
        sqp = ctx.enter_context(tc.tile_pool(name="sqp", bufs=2))
        statp = ctx.enter_context(tc.tile_pool(name="statp", bufs=3))
        hp = ctx.enter_context(tc.tile_pool(name="hp", bufs=2 * NTT))
        hTp = ctx.enter_context(tc.tile_pool(name="hTp", bufs=2))
        uTp = ctx.enter_context(tc.tile_pool(name="uTp", bufs=2))
        gTp = ctx.enter_context(tc.tile_pool(name="gTp", bufs=2))
        thTp = ctx.enter_context(tc.tile_pool(name="thTp", bufs=2))
        lamTp = ctx.enter_context(tc.tile_pool(name="lamTp", bufs=2))
        bTp = ctx.enter_context(tc.tile_pool(name="bTp", bufs=2))
        sTp = ctx.enter_context(tc.tile_pool(name="sTp", bufs=2))
        sgTp = ctx.enter_context(tc.tile_pool(name="sgTp", bufs=2))

        # ---- constants ----
        identf = const.tile([128, 128], f32, tag="identf", name="identf")
        masks.make_identity(nc, identf[:])
        identb = const.tile([128, 128], fr, tag="identb", name="identb")
        nc.gpsimd.tensor_copy(identb[:], identf[:])
        magic = const.tile([128, NTT], i32, tag="magic", name="magic")
        nc.gpsimd.memset(magic[:], MAGIC)

        def load_xb(c):
            """x chunk -> SBUF bf16 via two gpsimd SWDGE cast-DMAs."""
            xb = xp.tile([128, NTT * D], fr, tag="xb", name="xb")
            for half in range(2):
                lo = c * TC + half * 512
                src = x_d[lo:lo + 512, :].rearrange("(t p) d -> p t d", p=128)
                nc.gpsimd.dma_start(
                    xb[:, half * 4 * D:(half + 1) * 4 * D]
                    .rearrange("p (t d) -> p t d", d=D), src)
            return xb

        # prefetch the first two x chunks BEFORE the weight DMAs so the
        # compute-pipeline fill is not serialized behind them
        xbs = {c: load_xb(c) for c in range(min(2, NCHUNK))}

        # nw/bdt consts are needed late — issue them on the ACT queue so
        # the SP queue serves the weight DMAs (needed by the very first
        # PE transposes) with no head-of-line delay
        nw = []
        for k in range(2):
            t = const.tile([128, 1], f32, tag=f"nw{k}", name=f"nw{k}")
            nc.scalar.dma_start(t[:], nw_d[ts(k, 128)].rearrange("(a b) -> a b", b=1))
            nw.append(t)
        bdt2 = []
        for m in range(4):
            t = const.tile([128, 1], f32, tag=f"bdt{m}", name=f"bdt{m}")
            nc.scalar.dma_start(t[:], bdt_d[ts(m, 128)].rearrange("(a b) -> a b", b=1))
            # scale in place: tanh(pre*0.5 + b_dt*0.5)
            nc.vector.tensor_scalar_mul(t[:], t[:], 0.5)
            bdt2.append(t)

        # ---- load + transpose weights (scoped pools; DMAs split across
        # the SP and ACT HWDGE queues to halve the serial issue time) ----
        winT = [wT.tile([128, 1024], fr, tag=f"winT{k}", name=f"winT{k}")
                for k in range(2)]
        wdtT = [wT.tile([128, 512], fr, tag=f"wdtT{k}", name=f"wdtT{k}")
                for k in range(4)]
        woutT = [wT.tile([128, 256], fr, tag=f"woutT{e}", name=f"woutT{e}")
                 for e in range(4)]
        with tc.tile_pool(name="wraw", bufs=1) as wraw, \
                tc.tile_pool(name="wtps", bufs=2, space="PSUM") as wtps:
            # W_in [1024(feat), 256(d)] -> W_inT [2][128(d), 1024] * norm_w
            win_raw = []
            for f in range(8):
                t = wraw.tile([128, 256], f32, tag=f"winr{f}", name=f"winr{f}")
                # first 4 tiles feed the first transpose group: keep them
                # on SP (the ACT queue starts busy with the act-table load)
                eng = nc.sync if f < 4 or f % 2 == 0 else nc.scalar
                eng.dma_start(t[:], win_d[ts(f, 128), :])
                win_raw.append(t)
            for k in range(2):
                for half in range(2):
                    p = wtps.tile([128, 512], f32, tag="tpw", name="tpw")
                    for j in range(4):
                        f = half * 4 + j
                        nc.tensor.matmul(p[:, ts(j, 128)],
                                         win_raw[f][:, ts(k, 128)],
                                         identf[:], is_transpose=True)
                    if half % 2 == 0:
                        nc.vector.tensor_copy(winT[k][:, ts(half, 512)], p[:])
                    else:
                        nc.scalar.copy(winT[k][:, ts(half, 512)], p[:])
            for k in range(2):
                # fold norm_w (per-d row scale) into W_inT
                nc.vector.tensor_scalar_mul(winT[k][:], winT[k][:], nw[k][:])

            # W_dt [512(e_out), 512(e_in)] -> W_dtT [4][128(e_in), 512]
            wdt_raw = []
            for m in range(4):
                t = wraw.tile([128, 512], f32, tag=f"wdtr{m}", name=f"wdtr{m}")
                eng = nc.sync if m % 2 == 0 else nc.scalar
                eng.dma_start(t[:], wdt_d[ts(m, 128), :])
                wdt_raw.append(t)
            for k in range(4):
                p = wtps.tile([128, 512], f32, tag="tpw", name="tpw")
                for m in range(4):
                    nc.tensor.matmul(p[:, ts(m, 128)],
                                     wdt_raw[m][:, ts(k, 128)],
                                     identf[:], is_transpose=True)
                if k % 2 == 0:
                    nc.vector.tensor_copy(wdtT[k][:], p[:])
                else:
                    nc.scalar.copy(wdtT[k][:], p[:])

            # W_out [256(d), 512(e)] -> W_outT [4][128(e), 256(d)] * -0.5
            # (the -0.5 undoes the S = -2*s change of variable, so
            # sg = S*g can be a plain tensor mult)
            wout_raw = []
            for dd in range(2):
                t = wraw.tile([128, 512], f32, tag=f"woutr{dd}",
                              name=f"woutr{dd}")
                eng = nc.sync if dd % 2 == 0 else nc.scalar
                eng.dma_start(t[:], wout_d[ts(dd, 128), :])
                wout_raw.append(t)
            for e in range(4):
                p = wtps.tile([128, 512], f32, tag="tpw", name="tpw")
                for dd in range(2):
                    nc.tensor.matmul(p[:, ts(dd, 128)],
                                     wout_raw[dd][:, ts(e, 128)],
                                     identf[:], is_transpose=True)
                nc.vector.tensor_scalar_mul(woutT[e][:], p[:, :256], -0.5)

        # main-loop PSUM pools, entered after the weight-stage PSUM pool
        # closes (8 banks total: 2+2+2+2)
        tps = ctx.enter_context(tc.tile_pool(name="tps", bufs=2, space="PSUM"))
        uvps = ctx.enter_context(tc.tile_pool(name="uvps", bufs=2, space="PSUM"))
        preps = ctx.enter_context(tc.tile_pool(name="preps", bufs=2, space="PSUM"))
        yps = ctx.enter_context(tc.tile_pool(name="yps", bufs=2, space="PSUM"))

        def out_pair_ap(c, pair):
            lo = c * TC + pair * 256
            return out_d[lo:lo + 256, :].rearrange("(t p) d -> p t d", p=128)

        # ---- main chunk loop ----
        def norm_stage(st, c):
            """Load x chunk (bf16), RMS stats (Pool+DVE), Newton-rsqrt,
            h = x*inv_rms (Pool), PE-transpose -> hT."""
            xb = xbs.pop(c) if c in xbs else load_xb(c)

            # x^2 subtile sums: Pool multiply + DVE free-axis reduce
            # (tensor_tensor_reduce hangs TRN2 HW; ACT is silu/tanh-bound)
            vpk = statp.tile([128, NTT], f32, tag="vpk", name="vpk")
            for t in range(NTT):
                sq = sqp.tile([128, D], f32, tag="sq", name="sq")
                nc.gpsimd.tensor_mul(sq[:], xb[:, ts(t, D)], xb[:, ts(t, D)])
                nc.vector.tensor_reduce(vpk[:, t:t + 1], sq[:],
                                        axis=mybir.AxisListType.X,
                                        op=ALU.add)

            # inv_rms = rsqrt(vpk/D + eps) via Newton (nv + iters on Pool)
            nv = statp.tile([128, NTT], f32, tag="nv", name="nv")
            nc.gpsimd.tensor_scalar(nv[:], vpk[:], 1.0 / D, EPS,
                                    op0=ALU.mult, op1=ALU.add)
            ny = statp.tile([128, NTT], f32, tag="ny", name="ny")
            nt = statp.tile([128, NTT], f32, tag="nt", name="nt")
            # seed: y0 = bits(magic - (bits(v) >> 1)); MUST be the stt form
            # (a plain i32 tensor_sub runs through fp32 and loses low bits)
            nyi = ny[:].bitcast(i32)
            nc.vector.tensor_scalar(nyi, nv[:].bitcast(i32), 1, None,
                                    op0=ALU.arith_shift_right)
            nc.vector.scalar_tensor_tensor(nyi, magic[:], 1, nyi,
                                           op0=ALU.bypass, op1=ALU.subtract)
            for _ in range(NEWTON_ITERS):
                # t = v*y*y ; y = y * (1.5 - 0.5*t)
                nc.gpsimd.tensor_mul(nt[:], ny[:], ny[:])
                nc.gpsimd.tensor_mul(nt[:], nt[:], nv[:])
                nc.gpsimd.tensor_scalar(nt[:], nt[:], -0.5, 1.5,
                                        op0=ALU.mult, op1=ALU.add)
                nc.gpsimd.tensor_mul(ny[:], ny[:], nt[:])

            # h = x * inv_rms (bf16, Pool); PE-transpose into [128,1024]
            # bf16 PSUM banks (one per d-half); wide DVE 2x copies -> hT
            hT = hTp.tile([128, 2 * TC], fr, tag="hT", name="hT")
            hs = []
            for t in range(NTT):
                h = hp.tile([128, D], fr, tag="h", name="h")
                nc.gpsimd.tensor_scalar_mul(h[:], xb[:, ts(t, D)],
                                            ny[:, t:t + 1])
                hs.append(h)
            for k in range(2):
                p = tps.tile([128, TC], fr, tag="tp", name="tp")
                for t in range(NTT):
                    nc.tensor.matmul(p[:, ts(t, 128)], hs[t][:, ts(k, 128)],
                                     identb[:] if mm_f32r else identf[:],
                                     is_transpose=True)
                nc.vector.tensor_copy(hT[:, ts(k, TC)], p[:])
            st["hT"] = hT
            st["xb"] = xb

        def front_stage(st):
            """in_proj+silu, dt_proj+tanh, lam/tm (DVE), b (Pool)."""
            hT = st["hT"]
            uT = uTp.tile([128, WID], fr, tag="uT", name="uT")
            gT = gTp.tile([128, WID], fr, tag="gT", name="gT")
            for m in range(8):
                for th2 in range(2):
                    ps = uvps.tile([128, 512], f32, tag="uv", name="uv")
                    for k in range(2):
                        nc.tensor.matmul(
                            ps[:], winT[k][:, ts(m, 128)],
                            hT[:, k * TC + th2 * 512:k * TC + (th2 + 1) * 512],
                            start=(k == 0), stop=(k == 1),
                        )
                    off = (m % 4) * TC + th2 * 512
                    dst = (uT[:, off:off + 512] if m < 4
                           else gT[:, off:off + 512])
                    if silu_native:
                        nc.scalar.activation(dst, ps[:], AF.Silu)
                    else:
                        # CoreSim has no Silu: decompose as x * sigmoid(x)
                        sig = sqp.tile([128, 512], f32, tag="sig", name="sig")
                        nc.scalar.activation(sig[:], ps[:], AF.Sigmoid)
                        nc.vector.tensor_mul(dst, ps[:], sig[:])

            thT = thTp.tile([128, WID], fr, tag="thT", name="thT")
            for m in range(4):
                for th2 in range(2):
                    ps = preps.tile([128, 512], f32, tag="pre", name="pre")
                    for k in range(4):
                        nc.tensor.matmul(
                            ps[:], wdtT[k][:, ts(m, 128)],
                            uT[:, k * TC + th2 * 512:k * TC + (th2 + 1) * 512],
                            start=(k == 0), stop=(k == 3),
                        )
                    nc.scalar.activation(
                        thT[:, m * TC + th2 * 512:m * TC + (th2 + 1) * 512],
                        ps[:], AF.Tanh, bias=bdt2[m][:], scale=0.5)
            # lam = sigmoid(pre + b_dt) = 0.5*th + 0.5 and b = (th-1)*u,
            # per feature-quarter (4x-mode DVE ts + in-place Pool mult)
            # so scan(m) can start after 1/4 of the b-work, shortening
            # the serial tail of the last chunk
            lamT = lamTp.tile([128, WID], fr, tag="lamT", name="lamT")
            bT = bTp.tile([128, WID], fr, tag="bT", name="bT")
            for m in range(4):
                nc.vector.tensor_scalar(bT[:, ts(m, TC)], thT[:, ts(m, TC)],
                                        1.0, None, op0=ALU.subtract)
                nc.vector.tensor_scalar(lamT[:, ts(m, TC)],
                                        thT[:, ts(m, TC)], 0.5, 0.5,
                                        op0=ALU.mult, op1=ALU.add)
                nc.gpsimd.tensor_mul(bT[:, ts(m, TC)], bT[:, ts(m, TC)],
                                     uT[:, ts(m, TC)])
            st.update(uT=uT, gT=gT, lamT=lamT, bT=bT)

        def scan_stage(st, sT_prev):
            # S_t = lam_t*S_{t-1} + b_t (S = -2*s); 4 DVE scans into
            # slices of a wide sT; then ONE wide 2x-mode sg = S*g.
            sT = sTp.tile([128, WID], fr, tag="sT", name="sT")
            sgT = sgTp.tile([128, WID], fr, tag="sgT", name="sgT")
            for m in range(4):
                init = (0.0 if sT_prev is None
                        else sT_prev[:, (m + 1) * TC - 1:(m + 1) * TC])
                nc.vector.tensor_tensor_scan(
                    sT[:, ts(m, TC)], st["lamT"][:, ts(m, TC)],
                    st["bT"][:, ts(m, TC)], init,
                    op0=ALU.mult, op1=ALU.add,
                )
            # sg = S*g (the -0.5 is folded into woutT)
            nc.vector.tensor_mul(sgT[:], sT[:], st["gT"][:])
            st.update(sT=sT, sgT=sgT)
            return sT

        def out_stage(st, c):
            # out_proj -> PSUM, residual x added by the PE itself via an
            # identity-stationary matmul; plain ACT Copy drains; one wide
            # DMA out per 256 tokens.
            sgT, xb = st["sgT"], st["xb"]
            for pair in range(NTT // 2):
                yp = yps.tile([128, 512], f32, tag="y", name="y")
                tt = (2 * pair, 2 * pair + 1)
                for e in range(4):
                    for i, t in enumerate(tt):
                        # start=True zeroes the WHOLE 2KB psum bank, so
                        # only the first matmul into this tile sets it
                        nc.tensor.matmul(
                            yp[:, ts(i, D)], sgT[:, e * TC + t * 128:
                                                 e * TC + (t + 1) * 128],
                            woutT[e][:],
                            start=(e == 0 and i == 0), stop=False,
                            skip_group_check=True,
                        )
                # one [128,512]-wide identity matmul adds the residual x
                # for both subtiles of this pair at once
                nc.tensor.matmul(
                    yp[:], identb[:], xb[:, ts(pair, 512)],
                    start=False, stop=True, skip_group_check=True,
                )
                ot = outp.tile([128, 512], f32, tag="ot", name="ot")
                nc.scalar.copy(ot[:], yp[:])
                nc.sync.dma_start(
                    out_pair_ap(c, pair),
                    ot[:].rearrange("p (t d) -> p t d", d=D))

        # Software-pipelined emission. Engines run their streams in order,
        # so next chunk's PE-heavy front must be emitted BEFORE this
        # chunk's out_proj for PE to stay busy during the scan tail.
        for _ in range(repeat):
            sts = [dict() for _ in range(NCHUNK)]
            norm_stage(sts[0], 0)
            front_stage(sts[0])
            if NCHUNK > 1:
                norm_stage(sts[1], 1)
            sT_prev = None
            for c in range(NCHUNK):
                sT_prev = scan_stage(sts[c], sT_prev)
                if c + 1 < NCHUNK:
                    front_stage(sts[c + 1])
                if c + 2 < NCHUNK:
                    norm_stage(sts[c + 2], c + 2)
                out_stage(sts[c], c)
                sts[c].clear()


def build(mm_f32r=True, silu_native=True, repeat=1):
    """Build and compile the Bass module (cached)."""
    key = ("nc", mm_f32r, silu_native, repeat)
    if key in _CACHE:
        return _CACHE[key]

    from concourse import bacc, mybir, tile

    f32 = mybir.dt.float32
    nc = bacc.Bacc(
        "TRN2",
        target_bir_lowering=False,
        debug=False,
        num_devices=N_CORES,
    )
    aps = {
        "x": nc.dram_tensor("x", [K, D], f32, kind="ExternalInput").ap(),
        "norm_w": nc.dram_tensor("norm_w", [D], f32, kind="ExternalInput").ap(),
        "W_in": nc.dram_tensor("W_in", [2 * INNER, D], f32, kind="ExternalInput").ap(),
        "W_dt": nc.dram_tensor("W_dt", [INNER, INNER], f32, kind="ExternalInput").ap(),
        "b_dt": nc.dram_tensor("b_dt", [INNER], f32, kind="ExternalInput").ap(),
        "W_out": nc.dram_tensor("W_out", [D, INNER], f32, kind="ExternalInput").ap(),
        "out": nc.dram_tensor("out", [K, D], f32, kind="ExternalOutput").ap(),
    }
    with tile.TileContext(nc) as tc:
        _emit(tc, aps, mm_f32r=mm_f32r, silu_native=silu_native,
              repeat=repeat)
    nc.compile()
    _CACHE[key] = nc
    return nc


def make_in_maps(inputs):
    x = np.asarray(inputs["x"], dtype=np.float32)
    shared = {
        "norm_w": np.asarray(inputs["norm_w"], dtype=np.float32),
        "W_in": np.asarray(inputs["W_in"], dtype=np.float32),
        "W_dt": np.asarray(inputs["W_dt"], dtype=np.float32),
        "b_dt": np.asarray(inputs["b_dt"], dtype=np.float32),
        "W_out": np.asarray(inputs["W_out"], dtype=np.float32),
    }
    return [
        {"x": np.ascontiguousarray(x[b]), **shared} for b in range(N_CORES)
    ]


def run(inputs, trace=False, mm_f32r=True, silu_native=True, **kw):
    from concourse.bass_utils import run_bass_kernel_spmd

    nc = build(mm_f32r=mm_f32r, silu_native=silu_native)
    in_maps = make_in_maps(inputs)
    res = run_bass_kernel_spmd(
        nc, in_maps, core_ids=list(range(N_CORES)), trace=trace, **kw
    )
    out = np.stack([res.results[b]["out"] for b in range(N_CORES)], axis=0)
    return out, res


def kernel(**inputs) -> np.ndarray:
    out, _ = run(inputs, trace=False)
    return out
